# revision 1
# baseline (speedup 1.0000x reference)
"""Trainium2 Bass kernel for nn_Attention_50216757625003.

GQA attention layer: B=2, S=1024, D=4096, H=32 q-heads, KV=8 kv-heads,
hd=128, A=10 gated adapter tokens, RoPE, split softmax (adapter block
softmaxed separately and scaled by tanh(gate)), causal mask.

Sharding (8 NeuronCores): outer data-parallel over batch (2) x
tensor-parallel over heads (4 groups of 8 q-heads / 2 kv-heads).
wq/wk/wv are sharded column-wise, wo row-wise; each core computes a
partial [S, D] output contribution and the host sums the 4 head-group
partials per batch element.

Device-side layout tricks:
  * x is fed transposed ([D, S]) so all projections run with D on the
    contraction (partition) axis.
  * q/k head dims are permuted even-first on the host (wq/wk column
    permutation); RoPE pairs then live on partitions p and p+64.  A
    cheap SBUF->SBUF DMA swaps the halves so the rotation becomes four
    partition-aligned DVE ops against duplicated cos/sin tables.
  * scores are built transposed ([keys, q]) so softmax denominators come
    from a ones-vector matmul and probs feed the PV matmul directly (no
    transposes anywhere).
  * softmax skips the max-subtraction (scores are O(1) here; exp is safe
    in fp32), which the per-block normalization keeps exact.
  * matmuls default to bfloat16 operands (full PE rate at any tile size,
    half the HBM traffic of f32; rel err ~6e-3 vs the 2e-2 budget).
    KMM env switches "bf16"/"f32r"/"f32".
  * ONE psum pool spans all phases (tags: proj/vproj/av/ak, 8 banks
    exactly); avoiding pool close/reopen kills the cross-engine barrier
    stalls between projection, attention, and output phases.
  * the first K head's accumulation runs in 4-chunk blocks interleaved
    between V blocks, so x tiles stream in exactly one V-block ahead of
    every consumer and the PE never starves on the x DMA burst.
  * attention emits scores three chunk-groups ahead of the PV matmuls
    (each unit's last two key chunks share one psum tile and a single
    exp) and defers each unit's adapter-PV/normalization tail until
    after the next unit's prologue, hiding the scalar-engine exp
    latency and instruction overhead.
  * the causal diagonal is masked by a PE psum-accumulate of an
    additive -1e9 tile (identity lhsT), not a DVE multiply after exp.
  * softmax denominators live in one persistent psum bank (dt rows
    alternate partitions 0/64, gated adapter denominator at 32 with
    1/tanh(gate) folded into its contraction column); their reciprocals
    are partition-broadcast on the otherwise-idle GpSimd engine.
  * RoPE runs in bf16 (2x DVE) off the psum via an SBUF half-swap DMA;
    the psum-free copy moves to the DVE for the last two q-heads to
    keep the scalar engine clear for the first attention exps.
  * wo weights stream through a 4-deep ring DMA'd during attention;
    the last output tile is split to shorten the drain tail; the last
    q-head's RoPE is deferred past the first attention prologue.
"""

import os
import sys

import numpy as np

for _p in ("/opt/trn_rl_repo",):
    if _p not in sys.path and os.path.isdir(_p):
        sys.path.insert(0, _p)

import concourse.bass as bass
import concourse.mybir as mybir
from concourse import bacc
import concourse.tile as tile
from concourse.bass_utils import run_bass_kernel_spmd

HD = 128  # head dim (hardcoded: rope split + tile shapes assume 128)
A = 10    # adapter tokens
F32 = mybir.dt.float32

MM_MODE = os.environ.get("KMM", "bf16")

_PROG_CACHE = {}


def _md(mm):
    return {"f32r": mybir.dt.float32r, "f32": mybir.dt.float32,
            "bf16": mybir.dt.bfloat16}[mm]


# --------------------------------------------------------------------------
# device program
# --------------------------------------------------------------------------

def build_program(KO, S, HL, KVL, causal, mm):
    """One NeuronCore's program.

    KO: D // 128 contraction chunks.  S: sequence length.  HL: q heads on
    this core.  KVL: kv heads on this core.  causal: hardwire causal
    masking (tri mask on diagonal chunks + chunk skipping); otherwise an
    additive mask [S, S] is an input.  mm: matmul operand dtype mode.
    """
    nc = bacc.Bacc(None, target_bir_lowering=False,
                   dynamic_dma_scratch_size=2048)
    MD = _md(mm)
    D = KO * 128
    QB = min(512, S)       # q column block (psum bank + fp32 moving max)
    NQH = S // QB
    KC = S // 128          # token key chunks
    SA = S + A
    nrep = HL // KVL
    NB = D // 512          # wo column blocks
    NM = S // 128          # wo row chunks

    xp = nc.declare_dram_parameter("xp", [128, KO, S], MD, isOutput=False)
    wqp = nc.declare_dram_parameter("wqp", [HL, 128, KO, HD], MD, isOutput=False)
    wkp = nc.declare_dram_parameter("wkp", [KVL, 128, KO, HD], MD, isOutput=False)
    wvp = nc.declare_dram_parameter("wvp", [128, KO, KVL * HD], MD, isOutput=False)
    wop = nc.declare_dram_parameter("wop", [128, HL, D], MD, isOutput=False)
    adp = nc.declare_dram_parameter("adp", [128, KO, A], MD, isOutput=False)
    csp = nc.declare_dram_parameter("csp", [128, 2, S], MD, isOutput=False)
    trip = nc.declare_dram_parameter("trip", [128, 3, 128], MD, isOutput=False)
    ginvp = nc.declare_dram_parameter("ginvp", [128, HL], MD, isOutput=False)
    if not causal:
        mtp = nc.declare_dram_parameter("mtp", [128, KC, S], F32, isOutput=False)
    outp = nc.declare_dram_parameter("out", [NM, 128, D], F32, isOutput=True)

    Exp = mybir.ActivationFunctionType.Exp

    with tile.TileContext(nc) as tc:
        with tc.tile_pool(name="persist", bufs=1) as persist, \
             tc.tile_pool(name="wpool", bufs=4) as wpool, \
             tc.tile_pool(name="rpool", bufs=2) as rpool, \
             tc.tile_pool(name="cpool", bufs=1) as cpool, \
             tc.tile_pool(name="spool", bufs=1) as spool, \
             tc.tile_pool(name="obpool", bufs=4) as obpool, \
             tc.tile_pool(name="ps", bufs=1, space="PSUM") as ps:

            # resident x^T in XG-chunk tiles, DMA'd just-in-time from the
            # V-projection loop so the first matmuls start early
            XG = min(4, KO)
            NX = KO // XG
            xt = [persist.tile([128, XG, S], MD, tag=f"x{i}", name=f"x{i}")
                  for i in range(NX)]
            xt_loaded = [False] * NX

            def xload(i):
                if not xt_loaded[i]:
                    if i == 0:
                        # first chunk alone so the first matmul can start
                        # as soon as one chunk + one weight block land
                        nc.sync.dma_start(xt[i][:, 0:1, :], xp[:, 0:1, :])
                        if XG > 1:
                            nc.sync.dma_start(xt[i][:, 1:XG, :],
                                              xp[:, 1:XG, :])
                    else:
                        nc.sync.dma_start(xt[i],
                                          xp[:, i * XG:(i + 1) * XG, :])
                    xt_loaded[i] = True

            def xsl(c):
                return xt[c // XG][:, c % XG, :]

            kT = [persist.tile([128, SA], MD, tag=f"kT{j}", name=f"kT{j}")
                  for j in range(KVL)]
            vv = persist.tile([128, KC + 1, KVL * HD], MD, tag="vv")
            qT = [persist.tile([128, S], MD, tag=f"qT{h}", name=f"qT{h}")
                  for h in range(HL)]
            oT = [persist.tile([128, S], MD, tag=f"oT{h}", name=f"oT{h}")
                  for h in range(HL)]

            # cos/sin tables, adapter x^T, tanh(gate) row, tri mask
            csd = cpool.tile([128, 2, S], MD)
            adT = cpool.tile([128, KO, A], MD)
            ginv = cpool.tile([128, HL], MD)
            tri = cpool.tile([128, 3, 128], MD)
            vacc = cpool.tile([128, KC, KVL * HD], F32)
            zb = cpool.tile([128, 1], F32)

            # ---------------- phase 1: projections -----------------------
            pav = ps.tile([A, KVL * HD], F32, tag="av")

            WBV = min(4, KO)
            NVB = KO // WBV

            def emit_vblock(b):
                wt = wpool.tile([128, WBV, KVL * HD], MD, tag="w")
                nc.sync.dma_start(wt, wvp[:, b * WBV:(b + 1) * WBV, :])
                for i in range(b * WBV // XG,
                               (b * WBV + WBV - 1) // XG + 1):
                    xload(i)
                if b == 0:
                    nc.sync.dma_start(adT, adp[:])
                for t in range(KC):
                    psv = ps.tile([128, KVL * HD], F32, tag="vproj",
                                  bufs=2)
                    for ci in range(WBV):
                        c = b * WBV + ci
                        nc.tensor.matmul(
                            psv[:, :], xsl(c)[:, t * 128:(t + 1) * 128],
                            wt[:, ci, :],
                            start=(ci == 0), stop=(ci == WBV - 1))
                    if b == 0 and NVB > 1:
                        nc.scalar.copy(vacc[:, t, :], psv[:, :])
                    elif b < NVB - 1:
                        nc.vector.tensor_add(vacc[:, t, :], vacc[:, t, :],
                                             psv[:, :])
                    elif NVB > 1:
                        nc.vector.tensor_add(vv[:, t, :], vacc[:, t, :],
                                             psv[:, :])
                    else:
                        nc.scalar.copy(vv[:, t, :], psv[:, :])
                for ci in range(WBV):
                    c = b * WBV + ci
                    nc.tensor.matmul(pav[:, :], adT[:, c, :], wt[:, ci, :],
                                     start=(c == 0), stop=(c == KO - 1))
                if b == NVB - 1:
                    nc.scalar.copy(vv[0:A, KC, :], pav[:, :])
                if b == 0:
                    nc.vector.memset(zb, 0.0)
                    nc.sync.dma_start(csd, csp[:])
                    nc.sync.dma_start(tri, trip[:])
                    nc.sync.dma_start(ginv, ginvp[:])

            def emit_rope(ps_h, dst, hh, on_dve=False):
                # psum rows 0:64 = x0 (even pair elems), 64:128 = x1.
                # dst[0:64] = x0*cos - x1*sin ; dst[64:128] = x0*sin + x1*cos
                csA = csd[:, 0, :]
                csB = csd[:, 1, :]
                sl = slice(hh * QB, (hh + 1) * QB)
                rc = rpool.tile([128, QB], MD, tag="rc", bufs=2)
                if on_dve:
                    # keep the scalar engine free for attention exps near
                    # the phase transition
                    nc.vector.tensor_scalar_add(rc, ps_h, 0.0)
                else:
                    nc.scalar.copy(rc, ps_h)    # frees the psum slot fast
                rs = rpool.tile([128, QB], MD, tag="rs", bufs=2)
                nc.sync.dma_start(rs[0:64, :], rc[64:128, :])
                nc.sync.dma_start(rs[64:128, :], rc[0:64, :])
                # tm1 = [x0*cos ; x1*cos], tm2 = [x1*sin ; x0*sin]
                tm1 = rpool.tile([128, QB], MD, tag="tm1", bufs=2)
                tm2 = rpool.tile([128, QB], MD, tag="tm2", bufs=2)
                nc.vector.tensor_mul(tm1, rc, csA[:, sl])
                nc.vector.tensor_mul(tm2, rs, csB[:, sl])
                nc.vector.tensor_sub(dst[0:64, sl], tm1[0:64, :], tm2[0:64, :])
                nc.vector.tensor_add(dst[64:128, sl], tm2[64:128, :],
                                     tm1[64:128, :])

            def emit_head_block(hs, b, WB):
                """One WB-chunk accumulation block of a K (hs=('k',j)) or
                Q (hs=('q',h,psq)) head.  hs[1] indexes the weight param;
                hs[-1] is the [psum tiles, pak?] state made at block 0."""
                kind = hs[0]
                psq, pak = hs[-1]
                wt = wpool.tile([128, WB, HD], MD, tag="w")
                src = wkp if kind == "k" else wqp
                nc.sync.dma_start(wt, src[hs[1], :, b * WB:(b + 1) * WB, :])
                for i in range(b * WB // XG, (b * WB + WB - 1) // XG + 1):
                    xload(i)
                for ci in range(WB):
                    c = b * WB + ci
                    st, sp = (c == 0), (c == KO - 1)
                    for hh in range(NQH):
                        sl = slice(hh * QB, (hh + 1) * QB)
                        nc.tensor.matmul(
                            psq[hh][:, :], wt[:, ci, :], xsl(c)[:, sl],
                            start=st, stop=sp)
                    if pak is not None:
                        nc.tensor.matmul(
                            pak[:, :], wt[:, ci, :], adT[:, c, :],
                            start=st, stop=sp)

            def emit_head_finish(hs, on_dve=False):
                kind = hs[0]
                psq, pak = hs[-1]
                dst = kT[hs[1]] if kind == "k" else qT[hs[1]]
                for hh in range(NQH):
                    emit_rope(psq[hh], dst, hh, on_dve)
                if pak is not None:
                    nc.scalar.copy(dst[:, S:SA], pak[:, 0:A])

            def head_state(kind, idx):
                psq = [ps.tile([128, QB], F32, tag="proj", bufs=4,
                               name=f"ps_{kind}{idx}_{hh}")
                       for hh in range(NQH)]
                pak = ps.tile([128, A], F32, tag="ak",
                              name=f"pak{idx}") if kind == "k" else None
                return (kind, idx, (psq, pak))

            # Interleave the first K head's accumulation blocks (4 chunks
            # each) between V blocks so x tiles stream in exactly one
            # V-block ahead of every consumer, then run the remaining
            # heads with the full 8-chunk block size.
            emit_vblock(0)
            emit_vblock(1)
            k0 = head_state("k", 0)
            for b in range(KO // WBV):
                emit_head_block(k0, b, WBV)
                if b + 2 < NVB:
                    emit_vblock(b + 2)
            emit_head_finish(k0)
            WBQ = min(8, KO)
            deferred_rope = None
            for kind, idx in ([("k", j) for j in range(1, KVL)] +
                              [("q", h) for h in range(HL)]):
                hs = head_state(kind, idx)
                for b in range(KO // WBQ):
                    emit_head_block(hs, b, WBQ)
                if kind == "q" and idx == HL - 1 and HL > 1:
                    deferred_rope = hs   # finished after the first
                else:                    # attention prologue below
                    emit_head_finish(hs, on_dve=(kind == "q" and
                                                 idx >= HL - 2))

            # general (non-causal) mask tiles
            if not causal:
                mtt = [persist.tile([128, KC // 2, S], F32, tag=f"mt{i}",
                                    name=f"mt{i}")
                       for i in range(2)]
                nc.sync.dma_start(mtt[0], mtp[:, 0:KC // 2, :])
                nc.sync.dma_start(mtt[1], mtp[:, KC // 2:KC, :])

                def mtsl(kc):
                    return mtt[kc // (KC // 2)][:, kc % (KC // 2), :]

            # all-ones vectors: row 0 / column 127 of the tri mask
            ones_col = tri[:, 0, 127:128]
            eye = tri[:, 1, :]
            mneg = tri[:, 2, :]

            # ---------------- phase 2: attention --------------------------
            # Each unit's adapter PV + normalization tail is deferred
            # until after the NEXT unit's adapter scores + first three
            # score chunks, so the scalar engine is already working on
            # the next unit's exps while the PE drains the current tail.
            def emit_attn_prologue(h, qh):
                j = h // nrep
                qs, qe = qh * QB, (qh + 1) * QB
                if causal:
                    kcs = [kc for kc in range(KC) if kc * 128 < qe]
                else:
                    kcs = list(range(KC))
                # chunk widths; the last two chunks share one psum tile
                # and one exp instruction when they fit in a bank
                Ns = [qe - (max(qs, kc * 128) if causal else qs)
                      for kc in kcs]
                groups = [[ki] for ki in range(len(kcs))]
                if causal and len(kcs) >= 2 and Ns[-2] + Ns[-1] <= QB:
                    groups = groups[:-2] + [[len(kcs) - 2, len(kcs) - 1]]
                gof = {ki: gi for gi, g in enumerate(groups) for ki in g}
                st = {"h": h, "qh": qh, "j": j, "qs": qs, "qe": qe,
                      "kcs": kcs, "pts": {}, "groups": groups, "gof": gof}

                def emit_group(gi):
                    scp = ps.tile([128, QB], F32, tag="proj", bufs=4)
                    pt = spool.tile([128, QB], MD, tag="pt", bufs=4)
                    c0 = 0
                    for ki in groups[gi]:
                        kc = kcs[ki]
                        q0 = max(qs, kc * 128) if causal else qs
                        N = qe - q0
                        diag = causal and kc * 128 >= qs
                        nc.tensor.matmul(
                            scp[:, c0:c0 + N],
                            kT[j][:, kc * 128:(kc + 1) * 128],
                            qT[h][:, q0:qe], start=True, stop=not diag)
                        if diag:  # add -1e9 above the diagonal in psum
                            nc.tensor.matmul(scp[:, c0:c0 + 128], eye,
                                             mneg, start=False, stop=True)
                        if not causal:
                            sadd = spool.tile([128, QB], F32, tag="sadd",
                                              bufs=2)
                            nc.vector.tensor_add(
                                sadd[:, 0:N], scp[:, c0:c0 + N],
                                mtsl(kc)[:, q0:qe])
                            nc.scalar.activation(pt[:, c0:c0 + N],
                                                 sadd[:, 0:N], Exp,
                                                 bias=zb)
                        st["pts"][ki] = (pt, c0, q0, N)
                        c0 += N
                    if causal:
                        nc.scalar.activation(pt[:, 0:c0], scp[:, 0:c0],
                                             Exp, bias=zb)

                st["emit_group"] = emit_group
                st["next_g"] = min(3, len(groups))
                for gi in range(st["next_g"]):
                    emit_group(gi)
                # adapter scores after the token scores so their exp sits
                # behind the first token exps in the scalar-engine queue
                sca = ps.tile([128, QB], F32, tag="proj", bufs=4)
                nc.tensor.matmul(sca[0:A, :], kT[j][:, S:SA],
                                 qT[h][:, qs:qe], start=True, stop=True)
                pa = spool.tile([128, QB], MD, tag="pa", bufs=2)
                nc.scalar.activation(pa[0:A, :], sca[0:A, :], Exp,
                                     bias=zb[0:A, :])
                st["pa"] = pa
                return st

            # one persistent denominator bank: dt rows alternate between
            # partition 0 and 64 per unit (subtile WAR reaches two units
            # back), da always at 32 (consumed late, one unit of slack)
            dtda = ps.tile([128, QB], F32, tag="ak", name="dtda")

            def emit_attn_body(st, uidx):
                h, j, qs, kcs = st["h"], st["j"], st["qs"], st["kcs"]
                ot_ps = ps.tile([128, QB], F32, tag="vproj", bufs=2)
                oa_ps = ps.tile([128, QB], F32, tag="av", bufs=1)
                dr = 64 * (uidx % 2)
                st["ot_ps"], st["oa_ps"], st["dr"] = ot_ps, oa_ps, dr
                groups, gof = st["groups"], st["gof"]
                for ki, kc in enumerate(kcs):
                    pt, c0, q0, N = st["pts"].pop(ki)
                    s0, sp = (ki == 0), (ki == len(kcs) - 1)
                    nc.tensor.matmul(
                        ot_ps[:, q0 - qs:QB],
                        vv[:, kc, j * HD:(j + 1) * HD],
                        pt[:, c0:c0 + N], start=s0, stop=sp)
                    nc.tensor.matmul(
                        dtda[dr:dr + 1, q0 - qs:QB], ones_col[:, 0:1],
                        pt[:, c0:c0 + N], start=s0, stop=sp)
                    gi = gof[ki]
                    if ki == groups[gi][-1]:
                        # sca's slot frees after its exp, so the ring
                        # sustains four outstanding groups mid-unit
                        while (st["next_g"] < len(groups) and
                               st["next_g"] <= gi + 4):
                            st["emit_group"](st["next_g"])
                            st["next_g"] += 1
                # gated adapter denominator (1/tanh folded into the
                # contraction column, so its reciprocal is tanh(g)/da)
                nc.tensor.matmul(dtda[32:33, :], ginv[0:A, h:h + 1],
                                 st["pa"][0:A, :], start=True, stop=True)
                rt = spool.tile([1, QB], F32, tag="rt", bufs=2)
                nc.vector.reciprocal(rt, dtda[dr:dr + 1, :])
                ra = spool.tile([1, QB], F32, tag="ra", bufs=2)
                nc.vector.reciprocal(ra, dtda[32:33, :])
                rtb = spool.tile([128, QB], F32, tag="rtb", bufs=2)
                nc.gpsimd.partition_broadcast(rtb, rt[0:1, :])
                rab = spool.tile([128, QB], F32, tag="rab", bufs=2)
                nc.gpsimd.partition_broadcast(rab, ra[0:1, :])
                st["rtb"], st["rab"] = rtb, rab

            def emit_attn_tail(st):
                h, qh, j = st["h"], st["qh"], st["j"]
                qs, qe = st["qs"], st["qe"]
                pa, ot_ps, oa_ps = st["pa"], st["ot_ps"], st["oa_ps"]
                nc.tensor.matmul(oa_ps[:, :],
                                 vv[0:A, KC, j * HD:(j + 1) * HD],
                                 pa[0:A, :], start=True, stop=True)
                # oT = ot/denom_t + tanh(g)*oa/denom_a  (write-once)
                tq1 = spool.tile([128, QB], F32, tag="tq1", bufs=1)
                nc.vector.tensor_mul(tq1, ot_ps[:, :], st["rtb"])
                tq2 = spool.tile([128, QB], F32, tag="tq2", bufs=1)
                nc.vector.tensor_mul(tq2, oa_ps[:, :], st["rab"])
                nc.vector.tensor_add(oT[h][:, qs:qe], tq1, tq2)

            units = [(h, qh) for h in range(HL) for qh in range(NQH)]
            pending = None
            for uidx, (h, qh) in enumerate(units):
                st = emit_attn_prologue(h, qh)
                if deferred_rope is not None:
                    emit_head_finish(deferred_rope, on_dve=True)
                    deferred_rope = None
                if pending is not None:
                    emit_attn_tail(pending)
                emit_attn_body(st, uidx)
                pending = st
            emit_attn_tail(pending)

            # ---------------- phase 3: output projection ------------------
            # wo weights stream through a 3-deep ring, DMA'd up to three
            # blocks ahead of consumption.  The last output tile is split
            # into halves to shorten the drain tail.
            won = {}

            def load_won(n):
                if n < NB:
                    wt = wpool.tile([128, HL, 512], MD, tag="won", bufs=4,
                                    name=f"won{n}")
                    won[n] = wt
                    nc.sync.dma_start(
                        wt, wop[:, :, n * 512:(n + 1) * 512])

            for n in range(min(4, NB)):
                load_won(n)
            for n in range(NB):
                wt = won.pop(n)
                for m in range(NM):
                    last = (n == NB - 1) and (m == NM - 1)
                    halves = ((0, 256), (256, 512)) if last else ((0, 512),)
                    for c0, c1 in halves:
                        pso = ps.tile([128, 512], F32, tag="proj", bufs=4)
                        for hh in range(HL):
                            nc.tensor.matmul(
                                pso[:, 0:c1 - c0],
                                oT[hh][:, m * 128:(m + 1) * 128],
                                wt[:, hh, c0:c1],
                                start=(hh == 0), stop=(hh == HL - 1))
                        ob = obpool.tile([128, 512], F32, tag="ob")
                        nc.scalar.copy(ob[:, 0:c1 - c0], pso[:, 0:c1 - c0])
                        nc.sync.dma_start(
                            outp[m, :, n * 512 + c0:n * 512 + c1],
                            ob[:, 0:c1 - c0])
                load_won(n + 4)

    nc.compile()
    nc.finalize()
    return nc


def get_program(KO, S, HL, KVL, causal, mm):
    key = (KO, S, HL, KVL, causal, mm)
    if key not in _PROG_CACHE:
        _PROG_CACHE[key] = build_program(KO, S, HL, KVL, causal, mm)
    return _PROG_CACHE[key]


# --------------------------------------------------------------------------
# host-side sharding / layout prep
# --------------------------------------------------------------------------

_EVEN_FIRST = np.concatenate([np.arange(0, HD, 2), np.arange(1, HD, 2)])


def is_causal_mask(mask):
    S = mask.shape[-1]
    m = np.asarray(mask).reshape(S, S)
    iu = np.triu_indices(S, 1)
    il = np.tril_indices(S)
    return bool(np.all(m[il] == 0.0) and np.all(m[iu] <= -1e8))


def _np_md(mm):
    if mm == "bf16":
        import ml_dtypes
        return ml_dtypes.bfloat16
    return np.float32


def prep_core_inputs(core, G, x, wq, wk, wv, wo, adapter, gate,
                     freqs_cos, freqs_sin, mask, causal, mm=None):
    """Build the input dict for one core = (batch b, head-group g)."""
    mm = MM_MODE if mm is None else mm
    B, S, D = x.shape
    H = gate.shape[1]
    hd = wq.shape[1] // H
    KV = wk.shape[1] // hd
    KO = D // 128
    KC = S // 128
    HL, KVL = H // G, KV // G
    b, g = core // G, core % G
    hsl = slice(g * HL, (g + 1) * HL)
    ksl = slice(g * KVL, (g + 1) * KVL)
    idx = _EVEN_FIRST
    f32 = np.float32
    md = _np_md(mm)

    def c(a, dt=None):
        return np.ascontiguousarray(a, dtype=dt if dt is not None else md)

    xp = c(x[b].T.reshape(KO, 128, S).transpose(1, 0, 2))
    wq4 = wq.reshape(D, H, hd)[:, hsl][:, :, idx] * np.float32(1.0 / np.sqrt(hd))
    wqp = c(wq4.reshape(KO, 128, HL, hd).transpose(2, 1, 0, 3))
    wk4 = wk.reshape(D, KV, hd)[:, ksl][:, :, idx]
    wkp = c(wk4.reshape(KO, 128, KVL, hd).transpose(2, 1, 0, 3))
    wv4 = wv.reshape(D, KV, hd)[:, ksl]
    wvp = c(wv4.reshape(KO, 128, KVL * hd).transpose(1, 0, 2))
    wos = wo[g * HL * hd:(g + 1) * HL * hd]
    wop = c(wos.reshape(HL, hd, D).transpose(1, 0, 2))
    adp = c(adapter[0].T.reshape(KO, 128, A).transpose(1, 0, 2))
    # cos^T / sin^T, each duplicated across both partition halves
    ct = np.asarray(freqs_cos, dtype=f32).T      # [64, S]
    st = np.asarray(freqs_sin, dtype=f32).T
    csp = np.empty((128, 2, S), f32)
    csp[0:64, 0] = ct
    csp[64:128, 0] = ct
    csp[0:64, 1] = st
    csp[64:128, 1] = st
    csp = c(csp)
    triu = np.triu(np.ones((128, 128), dtype=f32))
    tri = np.empty((128, 3, 128), f32)
    tri[:, 0] = triu
    tri[:, 1] = np.eye(128, dtype=f32)
    tri[:, 2] = -1e9 * (1.0 - triu)
    tri = c(tri)
    gth = np.tanh(np.asarray(gate[0, hsl, 0, 0], dtype=np.float64)).astype(f32)
    with np.errstate(divide="ignore"):
        ginv = np.broadcast_to((1.0 / gth).reshape(1, HL), (128, HL))
    ginvp = c(ginv)
    inp = {"xp": xp, "wqp": wqp, "wkp": wkp, "wvp": wvp, "wop": wop,
           "adp": adp, "csp": csp, "trip": tri, "ginvp": ginvp}
    if not causal:
        mt = np.asarray(mask).reshape(S, S).T  # [keys, q]
        inp["mtp"] = c(mt.reshape(KC, 128, S).transpose(1, 0, 2), f32)
    return inp


# --------------------------------------------------------------------------
# entry point
# --------------------------------------------------------------------------

def kernel(x, wq, wk, wv, wo, adapter, gate, freqs_cos, freqs_sin, mask,
           _trace=False):
    x, wq, wk, wv, wo, adapter, gate, freqs_cos, freqs_sin, mask = (
        np.asarray(a) for a in
        (x, wq, wk, wv, wo, adapter, gate, freqs_cos, freqs_sin, mask))
    B, S, D = x.shape
    H = gate.shape[1]
    hd = wq.shape[1] // H
    KV = wk.shape[1] // hd
    G = 8 // B                      # head groups per batch over 8 cores
    HL, KVL = H // G, KV // G
    KO = D // 128

    causal = is_causal_mask(mask)
    nc = get_program(KO, S, HL, KVL, causal, MM_MODE)

    in_maps = [prep_core_inputs(core, G, x, wq, wk, wv, wo, adapter, gate,
                                freqs_cos, freqs_sin, mask, causal)
               for core in range(8)]
    res = run_bass_kernel_spmd(nc, in_maps, core_ids=list(range(8)),
                               trace=_trace)
    out = np.zeros((B, S, D), np.float32)
    for core in range(8):
        b = core // G
        r = res.results[core]
        out[b] += r["out"].reshape(S, D)
    if _trace:
        kernel._last_result = res
    return out



# revision 5
# speedup vs baseline: 1.1828x; 1.1828x over previous
"""Trainium2 Bass kernel for nn_Attention_50216757625003.

GQA attention layer: B=2, S=1024, D=4096, H=32 q-heads, KV=8 kv-heads,
hd=128, A=10 gated adapter tokens, RoPE, split softmax (adapter block
softmaxed separately and scaled by tanh(gate)), causal mask.

Sharding (8 NeuronCores): outer data-parallel over batch (2) x
tensor-parallel over heads (4 groups of 8 q-heads / 2 kv-heads).
wq/wk/wv are sharded column-wise, wo row-wise; each core computes a
partial [S, D] output contribution and the host sums the 4 head-group
partials per batch element.

Device-side layout tricks:
  * x is fed transposed ([D, S]) so all projections run with D on the
    contraction (partition) axis.
  * q/k head dims are permuted even-first on the host (wq/wk column
    permutation); RoPE pairs then live on partitions p and p+64.  A
    cheap SBUF->SBUF DMA swaps the halves so the rotation becomes four
    partition-aligned DVE ops against duplicated cos/sin tables.
  * scores are built transposed ([keys, q]) so softmax denominators come
    from a ones-vector matmul and probs feed the PV matmul directly (no
    transposes anywhere).
  * softmax skips the max-subtraction (scores are O(1) here; exp is safe
    in fp32), which the per-block normalization keeps exact.
  * default KMM=fp8 mode: the four big projections (Q/K/V/O) run as
    fp8e4 DoubleRow matmuls (0.5 cycles/row, two 128-deep k-tiles per
    instruction).  Every projection operand is split hi/lo into two
    e4m3 tensors sharing one power-of-2 scale (lo = exact residual of
    hi), and each product is evaluated with three terms
    (hi*hi + lo*hi + hi*lo), which restores bf16-level accuracy at
    0.75x the bf16 cycle count.  x / weights / adapter splits are
    precomputed on the host; the attention output is split on the fly
    (one extra scalar copy + DVE subtract per tile).  All scale
    factors fold into host-prepped constants: the RoPE tables absorb
    the Q/K psum descale, the softmax-denominator ones-column and the
    gate column absorb the V-path scale, and the output copy descales
    by a single immediate.  Attention itself (scores, exp, PV) stays
    bf16: the score contraction is only hd=128 so DoubleRow pairing
    would cost 1.5x, and fp8 probs would need an extra elementwise
    residual pass that the scalar engine can't absorb.
  * ONE psum pool spans all phases (tags: proj/vproj/av/ak, 8 banks
    exactly); avoiding pool close/reopen kills the cross-engine barrier
    stalls between projection, attention, and output phases.
  * the first K head's accumulation runs in 4-chunk blocks interleaved
    between V blocks, so x tiles stream in exactly one V-block ahead of
    every consumer and the PE never starves on the x DMA burst.
  * attention emits scores three chunk-groups ahead of the PV matmuls
    (each unit's last two key chunks share one psum tile and a single
    exp) and defers each unit's adapter-PV/normalization tail until
    after the next unit's prologue, hiding the scalar-engine exp
    latency and instruction overhead.
  * the causal diagonal is masked by a PE psum-accumulate of an
    additive -1e9 tile (identity lhsT), not a DVE multiply after exp.
  * softmax denominators live in one persistent psum bank (dt rows
    alternate partitions 0/64, gated adapter denominator at 32 with
    the V-scale/tanh(gate) factors folded into its contraction
    column); their reciprocals are partition-broadcast on the
    otherwise-idle GpSimd engine.
  * RoPE runs in bf16 (2x DVE) off the psum via an SBUF half-swap DMA;
    the psum-free copy moves to the DVE for the last two q-heads to
    keep the scalar engine clear for the first attention exps.
  * wo weights stream through a ring DMA'd during attention; the last
    output tile is split to shorten the drain tail; the last q-head's
    RoPE is deferred past the first attention prologue.
"""

import os
import sys

import numpy as np

for _p in ("/opt/trn_rl_repo",):
    if _p not in sys.path and os.path.isdir(_p):
        sys.path.insert(0, _p)

import concourse.bass as bass
import concourse.mybir as mybir
from concourse import bacc
import concourse.tile as tile
from concourse.bass_utils import run_bass_kernel_spmd

HD = 128  # head dim (hardcoded: rope split + tile shapes assume 128)
A = 10    # adapter tokens
AP2 = 16  # adapter dim padded so the DoubleRow pair step is 16-aligned
F32 = mybir.dt.float32
F8 = mybir.dt.float8e4

MM_MODE = os.environ.get("KMM", "fp8")

# fp8 power-of-2 scale plan (see module docstring):
#   x, adapter-x:    * 2^4      (sigma 16, max ~84 < 240)
#   wq (with 1/sqrt(hd) folded): * 2^14  (sigma ~22, max ~118)
#   wk, wv, wo:      * 2^10     (sigma 16, max ~83)
#   oT (runtime):    * 2^5      (|oT| <= ~5 -> max ~160)
AX = 2.0 ** 4
AWQ = 2.0 ** 14
AWK = 2.0 ** 10
AWV = 2.0 ** 10
AWO = 2.0 ** 10
GAMO = 2.0 ** 5
A1Q = AX * AWQ          # q psum scale
A1K = AX * AWK          # k psum scale
A1V = AX * AWV          # v path scale
UDT = A1V / GAMO        # ones-column value: dt_psum = UDT * sum(exp)
OSC = 1.0 / (GAMO * AWO)  # final output copy scale

_PROG_CACHE = {}


def _md(mm):
    return {"f32r": mybir.dt.float32r, "f32": mybir.dt.float32,
            "bf16": mybir.dt.bfloat16, "fp8": mybir.dt.bfloat16}[mm]


# --------------------------------------------------------------------------
# device program
# --------------------------------------------------------------------------

def build_program(KO, S, HL, KVL, causal, mm):
    """One NeuronCore's program.

    KO: D // 128 contraction chunks.  S: sequence length.  HL: q heads on
    this core.  KVL: kv heads on this core.  causal: hardwire causal
    masking (tri mask on diagonal chunks + chunk skipping); otherwise an
    additive mask [S, S] is an input.  mm: matmul operand dtype mode
    ("fp8" = DoubleRow fp8 projections + bf16 attention).
    """
    nc = bacc.Bacc(None, target_bir_lowering=False,
                   dynamic_dma_scratch_size=2048)
    MD = _md(mm)          # attention operand dtype (bf16 in fp8 mode)
    fp8 = mm == "fp8"
    WD = F8 if fp8 else MD  # projection operand dtype
    DR = mybir.MatmulPerfMode.DoubleRow if fp8 else None
    NV = 2 if fp8 else 1  # hi/lo variants per projection operand
    # product terms (x variant, w variant): hi*hi, lo*hi, hi*lo
    TERMS = ((0, 0), (1, 0), (0, 1)) if fp8 else ((0, 0),)
    D = KO * 128
    QB = min(512, S)       # q column block (psum bank + fp32 moving max)
    NQH = S // QB
    KC = S // 128          # token key chunks
    SA = S + A
    nrep = HL // KVL
    NB = D // 512          # wo column blocks
    NM = S // 128          # wo row chunks

    def dparam(name, shape, dt_):
        return nc.declare_dram_parameter(name, shape, dt_, isOutput=False)

    xp = [dparam(f"xp{v}", [128, KO, S], WD) for v in range(NV)]
    wqp = [dparam(f"wqp{v}", [HL, 128, KO, HD], WD) for v in range(NV)]
    wkp = [dparam(f"wkp{v}", [KVL, 128, KO, HD], WD) for v in range(NV)]
    wvp = [dparam(f"wvp{v}", [128, KO, KVL * HD], WD) for v in range(NV)]
    wop = [dparam(f"wop{v}", [128, HL, D], WD) for v in range(NV)]
    adp = [dparam(f"adp{v}", [128, KO, AP2], WD) for v in range(NV)]
    csp = dparam("csp", [128, 4, S], MD)
    trip = dparam("trip", [128, 3, 128], MD)
    ginvp = dparam("ginvp", [128, HL], MD)
    if not causal:
        mtp = dparam("mtp", [128, KC, S], F32)
    outp = nc.declare_dram_parameter("out", [NM, 128, D], F32, isOutput=True)

    Exp = mybir.ActivationFunctionType.Exp

    with tile.TileContext(nc) as tc:
        with tc.tile_pool(name="persist", bufs=1) as persist, \
             tc.tile_pool(name="wpool", bufs=4 * NV) as wpool, \
             tc.tile_pool(name="rpool", bufs=2) as rpool, \
             tc.tile_pool(name="cpool", bufs=1) as cpool, \
             tc.tile_pool(name="spool", bufs=1) as spool, \
             tc.tile_pool(name="obpool", bufs=4) as obpool, \
             tc.tile_pool(name="ps", bufs=1, space="PSUM") as ps:

            # resident x^T in XG-chunk tiles, DMA'd just-in-time from the
            # V-projection loop so the first matmuls start early
            XG = min(4, KO)
            NX = KO // XG
            xt = [[persist.tile([128, XG, S], WD, tag=f"x{v}_{i}",
                                name=f"x{v}_{i}")
                   for i in range(NX)] for v in range(NV)]
            xt_loaded = [False] * NX

            def xload(i):
                if not xt_loaded[i]:
                    if i == 0:
                        # first chunk alone so the first matmul can start
                        # as soon as one chunk + one weight block land
                        nc.sync.dma_start(xt[0][i][:, 0:1, :], xp[0][:, 0:1, :])
                        if XG > 1:
                            nc.sync.dma_start(xt[0][i][:, 1:XG, :],
                                              xp[0][:, 1:XG, :])
                        for v in range(1, NV):
                            nc.sync.dma_start(xt[v][i], xp[v][:, 0:XG, :])
                    else:
                        for v in range(NV):
                            nc.sync.dma_start(
                                xt[v][i], xp[v][:, i * XG:(i + 1) * XG, :])
                    xt_loaded[i] = True

            def xpair(c, v, sl):
                # [128, 2, sl] k-tile pair starting at chunk c (c even-offset)
                t_ = xt[v][c // XG]
                cc = c % XG
                return t_[:, cc:cc + 2, sl]

            def xsl(c, v=0):
                return xt[v][c // XG][:, c % XG, :]

            kT = [persist.tile([128, SA], MD, tag=f"kT{j}", name=f"kT{j}")
                  for j in range(KVL)]
            vv = persist.tile([128, KC + 1, KVL * HD], MD, tag="vv")
            qT = [persist.tile([128, S], MD, tag=f"qT{h}", name=f"qT{h}")
                  for h in range(HL)]
            if fp8:
                # attention output hi/lo fp8 (head dim packed for DoubleRow
                # head-pairing in the O projection)
                oT8 = [persist.tile([128, HL, S], F8, tag=f"oT8_{v}",
                                    name=f"oT8_{v}") for v in range(2)]
            else:
                oT = [persist.tile([128, S], MD, tag=f"oT{h}", name=f"oT{h}")
                      for h in range(HL)]

            # cos/sin tables (q rows 0-1, k rows 2-3), adapter x^T,
            # folded gate column, tri mask
            csd = cpool.tile([128, 4, S], MD)
            adT = [cpool.tile([128, KO, AP2], WD, name=f"adT{v}")
                   for v in range(NV)]
            ginv = cpool.tile([128, HL], MD)
            tri = cpool.tile([128, 3, 128], MD)
            vacc = cpool.tile([128, KC, KVL * HD], F32)
            zb = cpool.tile([128, 1], F32)

            # ---------------- phase 1: projections -----------------------
            pav = ps.tile([AP2, KVL * HD], F32, tag="av")

            WBV = min(4, KO)
            NVB = KO // WBV

            def emit_vblock(b):
                wt = [wpool.tile([128, WBV, KVL * HD], WD, tag="w",
                                 name=f"wtv{v}")
                      for v in range(NV)]
                for v in range(NV):
                    nc.sync.dma_start(wt[v], wvp[v][:, b * WBV:(b + 1) * WBV, :])
                for i in range(b * WBV // XG,
                               (b * WBV + WBV - 1) // XG + 1):
                    xload(i)
                if b == 0:
                    for v in range(NV):
                        nc.sync.dma_start(adT[v], adp[v][:])
                for t in range(KC):
                    psv = ps.tile([128, KVL * HD], F32, tag="vproj",
                                  bufs=2)
                    tsl = slice(t * 128, (t + 1) * 128)
                    if fp8:
                        n = 0
                        NT = len(TERMS) * (WBV // 2)
                        for xv, wv_ in TERMS:
                            for pi in range(WBV // 2):
                                c = b * WBV + 2 * pi
                                nc.tensor.matmul(
                                    psv[:, :], xpair(c, xv, tsl),
                                    wt[wv_][:, 2 * pi:2 * pi + 2, :],
                                    start=(n == 0), stop=(n == NT - 1),
                                    perf_mode=DR)
                                n += 1
                    else:
                        for ci in range(WBV):
                            c = b * WBV + ci
                            nc.tensor.matmul(
                                psv[:, :], xsl(c)[:, tsl], wt[0][:, ci, :],
                                start=(ci == 0), stop=(ci == WBV - 1))
                    if b == 0 and NVB > 1:
                        nc.scalar.copy(vacc[:, t, :], psv[:, :])
                    elif b < NVB - 1:
                        nc.vector.tensor_add(vacc[:, t, :], vacc[:, t, :],
                                             psv[:, :])
                    elif NVB > 1:
                        nc.vector.tensor_add(vv[:, t, :], vacc[:, t, :],
                                             psv[:, :])
                    else:
                        nc.scalar.copy(vv[:, t, :], psv[:, :])
                if fp8:
                    for ti, (xv, wv_) in enumerate(TERMS):
                        for pi in range(WBV // 2):
                            c = b * WBV + 2 * pi
                            nc.tensor.matmul(
                                pav[:, :], adT[xv][:, c:c + 2, :],
                                wt[wv_][:, 2 * pi:2 * pi + 2, :],
                                start=(b == 0 and ti == 0 and pi == 0),
                                stop=(b == NVB - 1 and ti == len(TERMS) - 1
                                      and pi == WBV // 2 - 1),
                                perf_mode=DR)
                else:
                    for ci in range(WBV):
                        c = b * WBV + ci
                        nc.tensor.matmul(pav[0:A, :], adT[0][:, c, 0:A],
                                         wt[0][:, ci, :],
                                         start=(c == 0), stop=(c == KO - 1))
                if b == NVB - 1:
                    nc.scalar.copy(vv[0:A, KC, :], pav[0:A, :])
                if b == 0:
                    nc.vector.memset(zb, 0.0)
                    nc.sync.dma_start(csd, csp[:])
                    nc.sync.dma_start(tri, trip[:])
                    nc.sync.dma_start(ginv, ginvp[:])

            def emit_rope(ps_h, dst, hh, tab, on_dve=False):
                # psum rows 0:64 = x0 (even pair elems), 64:128 = x1.
                # dst[0:64] = x0*cos - x1*sin ; dst[64:128] = x0*sin + x1*cos
                # (tables carry the 1/A1 psum descale in fp8 mode)
                csA = csd[:, tab, :]
                csB = csd[:, tab + 1, :]
                sl = slice(hh * QB, (hh + 1) * QB)
                rc = rpool.tile([128, QB], MD, tag="rc", bufs=2)
                if on_dve:
                    # keep the scalar engine free for attention exps near
                    # the phase transition
                    nc.vector.tensor_scalar_add(rc, ps_h, 0.0)
                else:
                    nc.scalar.copy(rc, ps_h)    # frees the psum slot fast
                rs = rpool.tile([128, QB], MD, tag="rs", bufs=2)
                nc.sync.dma_start(rs[0:64, :], rc[64:128, :])
                nc.sync.dma_start(rs[64:128, :], rc[0:64, :])
                # tm1 = [x0*cos ; x1*cos], tm2 = [x1*sin ; x0*sin]
                tm1 = rpool.tile([128, QB], MD, tag="tm1", bufs=2)
                tm2 = rpool.tile([128, QB], MD, tag="tm2", bufs=2)
                nc.vector.tensor_mul(tm1, rc, csA[:, sl])
                nc.vector.tensor_mul(tm2, rs, csB[:, sl])
                nc.vector.tensor_sub(dst[0:64, sl], tm1[0:64, :], tm2[0:64, :])
                nc.vector.tensor_add(dst[64:128, sl], tm2[64:128, :],
                                     tm1[64:128, :])

            def emit_head_block(hs, b, WB):
                """One WB-chunk accumulation block of a K (hs=('k',j)) or
                Q (hs=('q',h,psq)) head.  hs[1] indexes the weight param;
                hs[-1] is the [psum tiles, pak?] state made at block 0."""
                kind = hs[0]
                psq, pak = hs[-1]
                wt = [wpool.tile([128, WB, HD], WD, tag="w",
                                 name=f"wtk{v}")
                      for v in range(NV)]
                src = wkp if kind == "k" else wqp
                for v in range(NV):
                    nc.sync.dma_start(wt[v],
                                      src[v][hs[1], :, b * WB:(b + 1) * WB, :])
                for i in range(b * WB // XG, (b * WB + WB - 1) // XG + 1):
                    xload(i)
                NBK = KO // WB
                if fp8:
                    for pi in range(WB // 2):
                        c = b * WB + 2 * pi
                        for ti, (xv, wv_) in enumerate(TERMS):
                            st = (b == 0 and pi == 0 and ti == 0)
                            sp = (b == NBK - 1 and pi == WB // 2 - 1
                                  and ti == len(TERMS) - 1)
                            wpr = wt[wv_][:, 2 * pi:2 * pi + 2, :]
                            for hh in range(NQH):
                                sl = slice(hh * QB, (hh + 1) * QB)
                                nc.tensor.matmul(
                                    psq[hh][:, :], wpr, xpair(c, xv, sl),
                                    start=st, stop=sp, perf_mode=DR)
                            if pak is not None:
                                nc.tensor.matmul(
                                    pak[:, :], wpr, adT[xv][:, c:c + 2, :],
                                    start=st, stop=sp, perf_mode=DR)
                else:
                    for ci in range(WB):
                        c = b * WB + ci
                        st, sp = (c == 0), (c == KO - 1)
                        for hh in range(NQH):
                            sl = slice(hh * QB, (hh + 1) * QB)
                            nc.tensor.matmul(
                                psq[hh][:, :], wt[0][:, ci, :], xsl(c)[:, sl],
                                start=st, stop=sp)
                        if pak is not None:
                            nc.tensor.matmul(
                                pak[:, :], wt[0][:, ci, :], adT[0][:, c, :],
                                start=st, stop=sp)

            def emit_head_finish(hs, on_dve=False):
                kind = hs[0]
                psq, pak = hs[-1]
                dst = kT[hs[1]] if kind == "k" else qT[hs[1]]
                tab = 2 if kind == "k" else 0
                for hh in range(NQH):
                    emit_rope(psq[hh], dst, hh, tab, on_dve)
                if pak is not None:
                    if fp8:
                        nc.scalar.mul(dst[:, S:SA], pak[:, 0:A], 1.0 / A1K)
                    else:
                        nc.scalar.copy(dst[:, S:SA], pak[:, 0:A])

            def head_state(kind, idx):
                psq = [ps.tile([128, QB], F32, tag="proj", bufs=4,
                               name=f"ps_{kind}{idx}_{hh}")
                       for hh in range(NQH)]
                pak = ps.tile([128, AP2], F32, tag="ak",
                              name=f"pak{idx}") if kind == "k" else None
                return (kind, idx, (psq, pak))

            # Interleave the first K head's accumulation blocks (4 chunks
            # each) between V blocks so x tiles stream in exactly one
            # V-block ahead of every consumer, then run the remaining
            # heads with the full 8-chunk block size.
            emit_vblock(0)
            emit_vblock(1)
            k0 = head_state("k", 0)
            for b in range(KO // WBV):
                emit_head_block(k0, b, WBV)
                if b + 2 < NVB:
                    emit_vblock(b + 2)
            emit_head_finish(k0)
            WBQ = min(8, KO)
            deferred_rope = None
            for kind, idx in ([("k", j) for j in range(1, KVL)] +
                              [("q", h) for h in range(HL)]):
                hs = head_state(kind, idx)
                for b in range(KO // WBQ):
                    emit_head_block(hs, b, WBQ)
                if kind == "q" and idx == HL - 1 and HL > 1:
                    deferred_rope = hs   # finished after the first
                else:                    # attention prologue below
                    emit_head_finish(hs, on_dve=(kind == "q" and
                                                 idx >= HL - 2))

            # general (non-causal) mask tiles
            if not causal:
                mtt = [persist.tile([128, KC // 2, S], F32, tag=f"mt{i}",
                                    name=f"mt{i}")
                       for i in range(2)]
                nc.sync.dma_start(mtt[0], mtp[:, 0:KC // 2, :])
                nc.sync.dma_start(mtt[1], mtp[:, KC // 2:KC, :])

                def mtsl(kc):
                    return mtt[kc // (KC // 2)][:, kc % (KC // 2), :]

            # ones-column (value UDT in fp8 mode) / identity / -1e9 triangle
            ones_col = tri[:, 0, 127:128]
            eye = tri[:, 1, :]
            mneg = tri[:, 2, :]

            # ---------------- phase 2: attention --------------------------
            # Each unit's adapter PV + normalization tail is deferred
            # until after the NEXT unit's adapter scores + first three
            # score chunks, so the scalar engine is already working on
            # the next unit's exps while the PE drains the current tail.
            def emit_attn_prologue(h, qh):
                j = h // nrep
                qs, qe = qh * QB, (qh + 1) * QB
                if causal:
                    kcs = [kc for kc in range(KC) if kc * 128 < qe]
                else:
                    kcs = list(range(KC))
                # chunk widths; the last two chunks share one psum tile
                # and one exp instruction when they fit in a bank
                Ns = [qe - (max(qs, kc * 128) if causal else qs)
                      for kc in kcs]
                groups = [[ki] for ki in range(len(kcs))]
                if causal and len(kcs) >= 2 and Ns[-2] + Ns[-1] <= QB:
                    groups = groups[:-2] + [[len(kcs) - 2, len(kcs) - 1]]
                gof = {ki: gi for gi, g in enumerate(groups) for ki in g}
                st = {"h": h, "qh": qh, "j": j, "qs": qs, "qe": qe,
                      "kcs": kcs, "pts": {}, "groups": groups, "gof": gof}

                def emit_group(gi):
                    scp = ps.tile([128, QB], F32, tag="proj", bufs=4)
                    pt = spool.tile([128, QB], MD, tag="pt", bufs=4)
                    c0 = 0
                    for ki in groups[gi]:
                        kc = kcs[ki]
                        q0 = max(qs, kc * 128) if causal else qs
                        N = qe - q0
                        diag = causal and kc * 128 >= qs
                        nc.tensor.matmul(
                            scp[:, c0:c0 + N],
                            kT[j][:, kc * 128:(kc + 1) * 128],
                            qT[h][:, q0:qe], start=True, stop=not diag)
                        if diag:  # add -1e9 above the diagonal in psum
                            nc.tensor.matmul(scp[:, c0:c0 + 128], eye,
                                             mneg, start=False, stop=True)
                        if not causal:
                            sadd = spool.tile([128, QB], F32, tag="sadd",
                                              bufs=2)
                            nc.vector.tensor_add(
                                sadd[:, 0:N], scp[:, c0:c0 + N],
                                mtsl(kc)[:, q0:qe])
                            nc.scalar.activation(pt[:, c0:c0 + N],
                                                 sadd[:, 0:N], Exp,
                                                 bias=zb)
                        st["pts"][ki] = (pt, c0, q0, N)
                        c0 += N
                    if causal:
                        nc.scalar.activation(pt[:, 0:c0], scp[:, 0:c0],
                                             Exp, bias=zb)

                st["emit_group"] = emit_group
                st["next_g"] = min(3, len(groups))
                for gi in range(st["next_g"]):
                    emit_group(gi)
                # adapter scores after the token scores so their exp sits
                # behind the first token exps in the scalar-engine queue
                sca = ps.tile([128, QB], F32, tag="proj", bufs=4)
                nc.tensor.matmul(sca[0:A, :], kT[j][:, S:SA],
                                 qT[h][:, qs:qe], start=True, stop=True)
                pa = spool.tile([128, QB], MD, tag="pa", bufs=2)
                nc.scalar.activation(pa[0:A, :], sca[0:A, :], Exp,
                                     bias=zb[0:A, :])
                st["pa"] = pa
                return st

            # one persistent denominator bank: dt rows alternate between
            # partition 0 and 64 per unit (subtile WAR reaches two units
            # back), da always at 32 (consumed late, one unit of slack)
            dtda = ps.tile([128, QB], F32, tag="ak", name="dtda")

            def emit_attn_body(st, uidx):
                h, j, qs, kcs = st["h"], st["j"], st["qs"], st["kcs"]
                ot_ps = ps.tile([128, QB], F32, tag="vproj", bufs=2)
                oa_ps = ps.tile([128, QB], F32, tag="av", bufs=1)
                dr = 64 * (uidx % 2)
                st["ot_ps"], st["oa_ps"], st["dr"] = ot_ps, oa_ps, dr
                groups, gof = st["groups"], st["gof"]
                for ki, kc in enumerate(kcs):
                    pt, c0, q0, N = st["pts"].pop(ki)
                    s0, sp = (ki == 0), (ki == len(kcs) - 1)
                    nc.tensor.matmul(
                        ot_ps[:, q0 - qs:QB],
                        vv[:, kc, j * HD:(j + 1) * HD],
                        pt[:, c0:c0 + N], start=s0, stop=sp)
                    nc.tensor.matmul(
                        dtda[dr:dr + 1, q0 - qs:QB], ones_col[:, 0:1],
                        pt[:, c0:c0 + N], start=s0, stop=sp)
                    gi = gof[ki]
                    if ki == groups[gi][-1]:
                        # sca's slot frees after its exp, so the ring
                        # sustains four outstanding groups mid-unit
                        while (st["next_g"] < len(groups) and
                               st["next_g"] <= gi + 4):
                            st["emit_group"](st["next_g"])
                            st["next_g"] += 1
                # gated adapter denominator (UDT/tanh folded into the
                # contraction column)
                nc.tensor.matmul(dtda[32:33, :], ginv[0:A, h:h + 1],
                                 st["pa"][0:A, :], start=True, stop=True)
                rt = spool.tile([1, QB], F32, tag="rt", bufs=2)
                nc.vector.reciprocal(rt, dtda[dr:dr + 1, :])
                ra = spool.tile([1, QB], F32, tag="ra", bufs=2)
                nc.vector.reciprocal(ra, dtda[32:33, :])
                rtb = spool.tile([128, QB], F32, tag="rtb", bufs=2)
                nc.gpsimd.partition_broadcast(rtb, rt[0:1, :])
                rab = spool.tile([128, QB], F32, tag="rab", bufs=2)
                nc.gpsimd.partition_broadcast(rab, ra[0:1, :])
                st["rtb"], st["rab"] = rtb, rab

            def emit_attn_tail(st):
                h, qh, j = st["h"], st["qh"], st["j"]
                qs, qe = st["qs"], st["qe"]
                pa, ot_ps, oa_ps = st["pa"], st["ot_ps"], st["oa_ps"]
                nc.tensor.matmul(oa_ps[:, :],
                                 vv[0:A, KC, j * HD:(j + 1) * HD],
                                 pa[0:A, :], start=True, stop=True)
                # oT = ot/denom_t + tanh(g)*oa/denom_a  (write-once; the
                # fp8 path emits GAMO-scaled hi/lo e4m3 for the O proj)
                tq1 = spool.tile([128, QB], F32, tag="tq1", bufs=1)
                nc.vector.tensor_mul(tq1, ot_ps[:, :], st["rtb"])
                tq2 = spool.tile([128, QB], F32, tag="tq2", bufs=1)
                nc.vector.tensor_mul(tq2, oa_ps[:, :], st["rab"])
                if fp8:
                    tsum = spool.tile([128, QB], F32, tag="tsum", bufs=1)
                    nc.vector.tensor_add(tsum, tq1, tq2)
                    nc.scalar.copy(oT8[0][:, h, qs:qe], tsum)
                    nc.vector.tensor_sub(oT8[1][:, h, qs:qe], tsum,
                                         oT8[0][:, h, qs:qe])
                else:
                    nc.vector.tensor_add(oT[h][:, qs:qe], tq1, tq2)

            units = [(h, qh) for h in range(HL) for qh in range(NQH)]
            pending = None
            for uidx, (h, qh) in enumerate(units):
                st = emit_attn_prologue(h, qh)
                if deferred_rope is not None:
                    emit_head_finish(deferred_rope, on_dve=True)
                    deferred_rope = None
                if pending is not None:
                    emit_attn_tail(pending)
                emit_attn_body(st, uidx)
                pending = st
            emit_attn_tail(pending)

            # ---------------- phase 3: output projection ------------------
            # wo weights stream through a ring, DMA'd ahead of
            # consumption.  The last output tile is split into halves to
            # shorten the drain tail.
            won = {}

            def load_won(n):
                if n < NB:
                    wt = [wpool.tile([128, HL, 512], WD, tag="won",
                                     bufs=4 * NV, name=f"won{v}_{n}")
                          for v in range(NV)]
                    won[n] = wt
                    for v in range(NV):
                        nc.sync.dma_start(
                            wt[v], wop[v][:, :, n * 512:(n + 1) * 512])

            for n in range(min(4, NB)):
                load_won(n)
            for n in range(NB):
                wt = won.pop(n)
                for m in range(NM):
                    last = (n == NB - 1) and (m == NM - 1)
                    halves = ((0, 256), (256, 512)) if last else ((0, 512),)
                    for c0, c1 in halves:
                        pso = ps.tile([128, 512], F32, tag="proj", bufs=4)
                        msl = slice(m * 128, (m + 1) * 128)
                        if fp8:
                            n_ = 0
                            NT = len(TERMS) * (HL // 2)
                            for ov, wv_ in TERMS:
                                for hp in range(HL // 2):
                                    nc.tensor.matmul(
                                        pso[:, 0:c1 - c0],
                                        oT8[ov][:, 2 * hp:2 * hp + 2, msl],
                                        wt[wv_][:, 2 * hp:2 * hp + 2, c0:c1],
                                        start=(n_ == 0), stop=(n_ == NT - 1),
                                        perf_mode=DR)
                                    n_ += 1
                        else:
                            for hh in range(HL):
                                nc.tensor.matmul(
                                    pso[:, 0:c1 - c0],
                                    oT[hh][:, msl],
                                    wt[0][:, hh, c0:c1],
                                    start=(hh == 0), stop=(hh == HL - 1))
                        ob = obpool.tile([128, 512], F32, tag="ob")
                        if fp8:
                            nc.scalar.mul(ob[:, 0:c1 - c0],
                                          pso[:, 0:c1 - c0], OSC)
                        else:
                            nc.scalar.copy(ob[:, 0:c1 - c0],
                                           pso[:, 0:c1 - c0])
                        nc.sync.dma_start(
                            outp[m, :, n * 512 + c0:n * 512 + c1],
                            ob[:, 0:c1 - c0])
                load_won(n + 4)

    nc.compile()
    nc.finalize()
    return nc


def get_program(KO, S, HL, KVL, causal, mm):
    key = (KO, S, HL, KVL, causal, mm)
    if key not in _PROG_CACHE:
        _PROG_CACHE[key] = build_program(KO, S, HL, KVL, causal, mm)
    return _PROG_CACHE[key]


# --------------------------------------------------------------------------
# host-side sharding / layout prep
# --------------------------------------------------------------------------

_EVEN_FIRST = np.concatenate([np.arange(0, HD, 2), np.arange(1, HD, 2)])


def is_causal_mask(mask):
    S = mask.shape[-1]
    m = np.asarray(mask).reshape(S, S)
    iu = np.triu_indices(S, 1)
    il = np.tril_indices(S)
    return bool(np.all(m[il] == 0.0) and np.all(m[iu] <= -1e8))


def _np_md(mm):
    if mm in ("bf16", "fp8"):
        import ml_dtypes
        return ml_dtypes.bfloat16
    return np.float32


def _np_f8():
    import ml_dtypes
    return ml_dtypes.float8_e4m3


def _q8pair(a32):
    """Split scaled f32 array into e4m3 hi + exact-residual lo."""
    f8 = _np_f8()
    h = np.clip(a32, -240.0, 240.0).astype(f8)
    l = np.clip(a32 - h.astype(np.float32), -240.0, 240.0).astype(f8)
    return h, l


def prep_core_inputs(core, G, x, wq, wk, wv, wo, adapter, gate,
                     freqs_cos, freqs_sin, mask, causal, mm=None):
    """Build the input dict for one core = (batch b, head-group g)."""
    mm = MM_MODE if mm is None else mm
    fp8 = mm == "fp8"
    B, S, D = x.shape
    H = gate.shape[1]
    hd = wq.shape[1] // H
    KV = wk.shape[1] // hd
    KO = D // 128
    KC = S // 128
    HL, KVL = H // G, KV // G
    b, g = core // G, core % G
    hsl = slice(g * HL, (g + 1) * HL)
    ksl = slice(g * KVL, (g + 1) * KVL)
    idx = _EVEN_FIRST
    f32 = np.float32
    md = _np_md(mm)

    def c(a, dt=None):
        return np.ascontiguousarray(a, dtype=dt if dt is not None else md)

    def pairs(key, a32):
        if fp8:
            h, l = _q8pair(np.ascontiguousarray(a32, f32))
            return {key + "0": h, key + "1": l}
        return {key + "0": c(a32)}

    inp = {}
    xp = x[b].T.reshape(KO, 128, S).transpose(1, 0, 2)
    inp.update(pairs("xp", xp * AX if fp8 else xp))
    wq4 = wq.reshape(D, H, hd)[:, hsl][:, :, idx] * np.float32(1.0 / np.sqrt(hd))
    wq4 = wq4.reshape(KO, 128, HL, hd).transpose(2, 1, 0, 3)
    inp.update(pairs("wqp", wq4 * AWQ if fp8 else wq4))
    wk4 = wk.reshape(D, KV, hd)[:, ksl][:, :, idx]
    wk4 = wk4.reshape(KO, 128, KVL, hd).transpose(2, 1, 0, 3)
    inp.update(pairs("wkp", wk4 * AWK if fp8 else wk4))
    wv4 = wv.reshape(D, KV, hd)[:, ksl]
    wv4 = wv4.reshape(KO, 128, KVL * hd).transpose(1, 0, 2)
    inp.update(pairs("wvp", wv4 * AWV if fp8 else wv4))
    wos = wo[g * HL * hd:(g + 1) * HL * hd]
    wos = wos.reshape(HL, hd, D).transpose(1, 0, 2)
    inp.update(pairs("wop", wos * AWO if fp8 else wos))
    adx = adapter[0].T.reshape(KO, 128, A).transpose(1, 0, 2)
    adxp = np.zeros((128, KO, AP2), f32)
    adxp[:, :, :A] = adx
    inp.update(pairs("adp", adxp * AX if fp8 else adxp))
    # cos^T / sin^T, duplicated across both partition halves; q rows 0-1
    # carry the q-psum descale, k rows 2-3 the k-psum descale (fp8 mode)
    ct = np.asarray(freqs_cos, dtype=f32).T      # [64, S]
    st = np.asarray(freqs_sin, dtype=f32).T
    sq = 1.0 / A1Q if fp8 else 1.0
    sk = 1.0 / A1K if fp8 else 1.0
    csp = np.empty((128, 4, S), f32)
    for half in (slice(0, 64), slice(64, 128)):
        csp[half, 0] = ct * sq
        csp[half, 1] = st * sq
        csp[half, 2] = ct * sk
        csp[half, 3] = st * sk
    csp = c(csp)
    triu = np.triu(np.ones((128, 128), dtype=f32))
    tri = np.empty((128, 3, 128), f32)
    tri[:, 0] = (UDT if fp8 else 1.0) * np.ones((128, 128), f32)
    tri[:, 1] = np.eye(128, dtype=f32)
    tri[:, 2] = -1e9 * (1.0 - triu)
    tri = c(tri)
    gth = np.tanh(np.asarray(gate[0, hsl, 0, 0], dtype=np.float64)).astype(f32)
    with np.errstate(divide="ignore"):
        gi = (UDT if fp8 else 1.0) / gth
        ginv = np.broadcast_to(gi.reshape(1, HL), (128, HL))
    ginvp = c(ginv)
    inp.update({"csp": csp, "trip": tri, "ginvp": ginvp})
    if not causal:
        mt = np.asarray(mask).reshape(S, S).T  # [keys, q]
        inp["mtp"] = c(mt.reshape(KC, 128, S).transpose(1, 0, 2), f32)
    return inp


# --------------------------------------------------------------------------
# entry point
# --------------------------------------------------------------------------

def kernel(x, wq, wk, wv, wo, adapter, gate, freqs_cos, freqs_sin, mask,
           _trace=False):
    x, wq, wk, wv, wo, adapter, gate, freqs_cos, freqs_sin, mask = (
        np.asarray(a) for a in
        (x, wq, wk, wv, wo, adapter, gate, freqs_cos, freqs_sin, mask))
    B, S, D = x.shape
    H = gate.shape[1]
    hd = wq.shape[1] // H
    KV = wk.shape[1] // hd
    G = 8 // B                      # head groups per batch over 8 cores
    HL, KVL = H // G, KV // G
    KO = D // 128

    causal = is_causal_mask(mask)
    nc = get_program(KO, S, HL, KVL, causal, MM_MODE)

    in_maps = [prep_core_inputs(core, G, x, wq, wk, wv, wo, adapter, gate,
                                freqs_cos, freqs_sin, mask, causal)
               for core in range(8)]
    res = run_bass_kernel_spmd(nc, in_maps, core_ids=list(range(8)),
                               trace=_trace)
    out = np.zeros((B, S, D), np.float32)
    for core in range(8):
        b = core // G
        r = res.results[core]
        out[b] += r["out"].reshape(S, D)
    if _trace:
        kernel._last_result = res
    return out


# revision 33
# speedup vs baseline: 1.1877x; 1.0041x over previous
"""Trainium2 Bass kernel for nn_Attention_50216757625003.

GQA attention layer: B=2, S=1024, D=4096, H=32 q-heads, KV=8 kv-heads,
hd=128, A=10 gated adapter tokens, RoPE, split softmax (adapter block
softmaxed separately and scaled by tanh(gate)), causal mask.

Sharding (8 NeuronCores): outer data-parallel over batch (2) x
tensor-parallel over heads (4 groups of 8 q-heads / 2 kv-heads).
wq/wk/wv are sharded column-wise, wo row-wise; each core computes a
partial [S, D] output contribution and the host sums the 4 head-group
partials per batch element.

Device-side layout tricks:
  * x is fed transposed ([D, S]) so all projections run with D on the
    contraction (partition) axis.
  * q/k head dims are permuted even-first on the host (wq/wk column
    permutation); RoPE pairs then live on partitions p and p+64.  A
    cheap SBUF->SBUF DMA swaps the halves so the rotation becomes four
    partition-aligned DVE ops against duplicated cos/sin tables.
  * scores are built transposed ([keys, q]) so softmax denominators come
    from a ones-vector matmul and probs feed the PV matmul directly (no
    transposes anywhere).
  * softmax skips the max-subtraction (scores are O(1) here; exp is safe
    in fp32), which the per-block normalization keeps exact.
  * default KMM=fp8 mode: the four big projections (Q/K/V/O) run as
    fp8e4 DoubleRow matmuls (0.5 cycles/row, two 128-deep k-tiles per
    instruction).  Every projection operand is split hi/lo into two
    e4m3 tensors sharing one power-of-2 scale (lo = exact residual of
    hi), and each product is evaluated with three terms
    (hi*hi + lo*hi + hi*lo), which restores bf16-level accuracy at
    0.75x the bf16 cycle count.  x / weights / adapter splits are
    precomputed on the host; the attention output is split on the fly
    (one extra scalar copy + DVE subtract per tile).  All scale
    factors fold into host-prepped constants: the RoPE tables absorb
    the Q/K psum descale (4-row table: q rows then k rows), the
    softmax-denominator ones-column and the gate column absorb the
    V-path scale, and the output copy descales by a single immediate.
    Attention itself (scores, exp, PV) stays bf16: the score
    contraction is only hd=128 so DoubleRow pairing would cost 1.5x,
    and fp8 probs would need an extra elementwise residual pass that
    the scalar engine can't absorb.  The adapter dim is zero-padded
    10->16 so DoubleRow pair strides stay 16-byte aligned, both kv
    heads' adapter-K psums share one bank (disjoint column ranges),
    and output partials leave the core as bf16 (the host sums in
    fp32), halving output DMA in the DMA-tight O-projection phase.
  * ONE psum pool spans all phases (tags: proj/vproj/av/ak, 8 banks
    exactly); avoiding pool close/reopen kills the cross-engine barrier
    stalls between projection, attention, and output phases.
  * the first K head's accumulation runs in 4-chunk blocks interleaved
    between V blocks, so x tiles stream in exactly one V-block ahead of
    every consumer and the PE never starves on the x DMA burst.
  * attention emits scores three chunk-groups ahead of the PV matmuls
    (each unit's last two key chunks share one psum tile and a single
    exp) and defers each unit's adapter-PV/normalization tail until
    after the next unit's prologue, hiding the scalar-engine exp
    latency and instruction overhead.
  * the causal diagonal is masked by a PE psum-accumulate of an
    additive -1e9 tile (identity lhsT), not a DVE multiply after exp.
  * softmax denominators live in one persistent psum bank (dt rows
    alternate partitions 0/64, gated adapter denominator at 32 with
    the V-scale/tanh(gate) factors folded into its contraction
    column); their reciprocals are partition-broadcast on the
    otherwise-idle GpSimd engine.
  * RoPE runs in bf16 (2x DVE) off the psum via an SBUF half-swap DMA;
    the psum-free copy moves to the DVE for the last two q-heads to
    keep the scalar engine clear for the first attention exps.
  * wo weights stream through a ring DMA'd during attention; the last
    output tile is split to shorten the drain tail; the last q-head's
    RoPE is deferred past the first attention prologue.
"""

import os
import sys

import numpy as np

for _p in ("/opt/trn_rl_repo",):
    if _p not in sys.path and os.path.isdir(_p):
        sys.path.insert(0, _p)

import concourse.bass as bass
import concourse.mybir as mybir
from concourse import bacc
import concourse.tile as tile
from concourse.bass_utils import run_bass_kernel_spmd

HD = 128  # head dim (hardcoded: rope split + tile shapes assume 128)
A = 10    # adapter tokens
AP2 = 16  # adapter dim padded so the DoubleRow pair step is 16-aligned
F32 = mybir.dt.float32
F8 = mybir.dt.float8e4

MM_MODE = os.environ.get("KMM", "fp8")

# fp8 power-of-2 scale plan (see module docstring):
#   x, adapter-x:    * 2^4      (sigma 16, max ~84 < 240)
#   wq (with 1/sqrt(hd) folded): * 2^14  (sigma ~22, max ~118)
#   wk, wv, wo:      * 2^10     (sigma 16, max ~83)
#   oT (runtime):    * 2^5      (|oT| <= ~5 -> max ~160)
AX = 2.0 ** 4
AWQ = 2.0 ** 14
AWK = 2.0 ** 10
AWV = 2.0 ** 10
AWO = 2.0 ** 10
GAMO = 2.0 ** 5
A1Q = AX * AWQ          # q psum scale
A1K = AX * AWK          # k psum scale
A1V = AX * AWV          # v path scale
UDT = A1V / GAMO        # ones-column value: dt_psum = UDT * sum(exp)
OSC = 1.0 / (GAMO * AWO)  # final output copy scale

_PROG_CACHE = {}


def _md(mm):
    return {"f32r": mybir.dt.float32r, "f32": mybir.dt.float32,
            "bf16": mybir.dt.bfloat16, "fp8": mybir.dt.bfloat16}[mm]


# --------------------------------------------------------------------------
# device program
# --------------------------------------------------------------------------

def build_program(KO, S, HL, KVL, causal, mm):
    """One NeuronCore's program.

    KO: D // 128 contraction chunks.  S: sequence length.  HL: q heads on
    this core.  KVL: kv heads on this core.  causal: hardwire causal
    masking (tri mask on diagonal chunks + chunk skipping); otherwise an
    additive mask [S, S] is an input.  mm: matmul operand dtype mode
    ("fp8" = DoubleRow fp8 projections + bf16 attention).
    """
    nc = bacc.Bacc(None, target_bir_lowering=False,
                   dynamic_dma_scratch_size=2048)
    MD = _md(mm)          # attention operand dtype (bf16 in fp8 mode)
    fp8 = mm == "fp8"
    WD = F8 if fp8 else MD  # projection operand dtype
    DR = mybir.MatmulPerfMode.DoubleRow if fp8 else None
    NV = 2 if fp8 else 1  # hi/lo variants per projection operand
    # product terms (x variant, w variant): hi*hi, lo*hi, hi*lo
    TERMS = ((0, 0), (1, 0), (0, 1)) if fp8 else ((0, 0),)
    D = KO * 128
    QB = min(512, S)       # q column block (psum bank + fp32 moving max)
    NQH = S // QB
    KC = S // 128          # token key chunks
    SA = S + A
    nrep = HL // KVL
    NB = D // 512          # wo column blocks
    NM = S // 128          # wo row chunks

    def dparam(name, shape, dt_):
        return nc.declare_dram_parameter(name, shape, dt_, isOutput=False)

    # hi/lo variants packed on one axis right after the partition dim:
    # one DMA per block loads both, and the innermost runs stay >= 512 B
    xp = dparam("xp", [128, NV, KO, S], WD)
    wqp = dparam("wqp", [HL, 128, NV, KO, HD], WD)
    wkp = dparam("wkp", [KVL, 128, NV, KO, HD], WD)
    wvp = dparam("wvp", [128, NV, KO, KVL * HD], WD)
    wop = dparam("wop", [128, NV, HL, D], WD)
    adp = dparam("adp", [128, NV, KO, AP2], WD)
    csp = dparam("csp", [128, 4, S], MD)
    trip = dparam("trip", [128, 3, 128], MD)
    ginvp = dparam("ginvp", [128, HL], MD)
    if not causal:
        mtp = dparam("mtp", [128, KC, S], F32)
    outp = nc.declare_dram_parameter("out", [NM, 128, D],
                                     MD if fp8 else F32, isOutput=True)

    Exp = mybir.ActivationFunctionType.Exp

    with tile.TileContext(nc) as tc:
        with tc.tile_pool(name="persist", bufs=1) as persist, \
             tc.tile_pool(name="wpool", bufs=4) as wpool, \
             tc.tile_pool(name="rpool", bufs=2) as rpool, \
             tc.tile_pool(name="cpool", bufs=1) as cpool, \
             tc.tile_pool(name="spool", bufs=1) as spool, \
             tc.tile_pool(name="obpool", bufs=4) as obpool, \
             tc.tile_pool(name="ps", bufs=1, space="PSUM") as ps:

            # resident x^T in XG-chunk tiles, DMA'd just-in-time from the
            # V-projection loop so the first matmuls start early
            XG = min(4, KO)
            NX = KO // XG
            xt = [persist.tile([128, NV, XG, S], WD, tag=f"x_{i}",
                               name=f"x_{i}")
                  for i in range(NX)]
            xt_loaded = [False] * NX

            def xload(i):
                if not xt_loaded[i]:
                    if i == 0:
                        # first hi k-pair alone so the first matmul can
                        # start as soon as one k-pair + one weight block land
                        c1 = min(2, XG)
                        nc.sync.dma_start(xt[i][:, 0:1, 0:c1, :],
                                          xp[:, 0:1, 0:c1, :])
                        if XG > c1:
                            nc.sync.dma_start(xt[i][:, 0:1, c1:XG, :],
                                              xp[:, 0:1, c1:XG, :])
                        if NV > 1:
                            nc.sync.dma_start(xt[i][:, 1:NV, :, :],
                                              xp[:, 1:NV, 0:XG, :])
                    else:
                        nc.sync.dma_start(
                            xt[i], xp[:, :, i * XG:(i + 1) * XG, :])
                    xt_loaded[i] = True

            def xpair(c, v, sl):
                # [128, 2, sl] k-tile pair starting at chunk c (c even-offset)
                t_ = xt[c // XG]
                cc = c % XG
                return t_[:, v, cc:cc + 2, sl]

            def xsl(c):
                return xt[c // XG][:, 0, c % XG, :]

            kT = [persist.tile([128, SA], MD, tag=f"kT{j}", name=f"kT{j}")
                  for j in range(KVL)]
            vv = persist.tile([128, KC + 1, KVL * HD], MD, tag="vv")
            qT = [persist.tile([128, S], MD, tag=f"qT{h}", name=f"qT{h}")
                  for h in range(HL)]
            if fp8:
                # attention output hi/lo fp8 (head dim packed for DoubleRow
                # head-pairing in the O projection)
                oT8 = persist.tile([128, 2, HL, S], F8, tag="oT8")
            else:
                oT = [persist.tile([128, S], MD, tag=f"oT{h}", name=f"oT{h}")
                      for h in range(HL)]

            # cos/sin tables (q rows 0-1, k rows 2-3), adapter x^T,
            # folded gate column, tri mask
            csd = cpool.tile([128, 4, S], MD)
            adT = cpool.tile([128, NV, KO, AP2], WD)
            ginv = cpool.tile([128, HL], MD)
            tri = cpool.tile([128, 3, 128], MD)
            vacc = cpool.tile([128, KC, KVL * HD], F32)
            zb = cpool.tile([128, 1], F32)

            # ---------------- phase 1: projections -----------------------
            pav = ps.tile([AP2, KVL * HD], F32, tag="av")

            WBV = min(4, KO)
            NVB = KO // WBV

            def emit_vblock(b):
                wt = wpool.tile([128, NV, WBV, KVL * HD], WD, tag="w",
                                name="wtv")
                nc.sync.dma_start(wt, wvp[:, :, b * WBV:(b + 1) * WBV, :])
                for i in range(b * WBV // XG,
                               (b * WBV + WBV - 1) // XG + 1):
                    xload(i)
                if b == 0:
                    nc.sync.dma_start(adT, adp[:])
                for t in range(KC):
                    psv = ps.tile([128, KVL * HD], F32, tag="vproj",
                                  bufs=2)
                    tsl = slice(t * 128, (t + 1) * 128)
                    if fp8:
                        n = 0
                        NT = len(TERMS) * (WBV // 2)
                        for xv, wv_ in TERMS:
                            for pi in range(WBV // 2):
                                c = b * WBV + 2 * pi
                                nc.tensor.matmul(
                                    psv[:, :], xpair(c, xv, tsl),
                                    wt[:, wv_, 2 * pi:2 * pi + 2, :],
                                    start=(n == 0), stop=(n == NT - 1),
                                    perf_mode=DR)
                                n += 1
                    else:
                        for ci in range(WBV):
                            c = b * WBV + ci
                            nc.tensor.matmul(
                                psv[:, :], xsl(c)[:, tsl], wt[:, 0, ci, :],
                                start=(ci == 0), stop=(ci == WBV - 1))
                    if b == 0 and NVB > 1:
                        nc.scalar.copy(vacc[:, t, :], psv[:, :])
                    elif b < NVB - 1:
                        nc.vector.tensor_add(vacc[:, t, :], vacc[:, t, :],
                                             psv[:, :])
                    elif NVB > 1:
                        nc.vector.tensor_add(vv[:, t, :], vacc[:, t, :],
                                             psv[:, :])
                    else:
                        nc.scalar.copy(vv[:, t, :], psv[:, :])
                if fp8:
                    for ti, (xv, wv_) in enumerate(TERMS):
                        for pi in range(WBV // 2):
                            c = b * WBV + 2 * pi
                            nc.tensor.matmul(
                                pav[:, :], adT[:, xv, c:c + 2, :],
                                wt[:, wv_, 2 * pi:2 * pi + 2, :],
                                start=(b == 0 and ti == 0 and pi == 0),
                                stop=(b == NVB - 1 and ti == len(TERMS) - 1
                                      and pi == WBV // 2 - 1),
                                perf_mode=DR)
                else:
                    for ci in range(WBV):
                        c = b * WBV + ci
                        nc.tensor.matmul(pav[0:A, :], adT[:, 0, c, 0:A],
                                         wt[:, 0, ci, :],
                                         start=(c == 0), stop=(c == KO - 1))
                if b == NVB - 1:
                    nc.scalar.copy(vv[0:A, KC, :], pav[0:A, :])
                if b == 0:
                    nc.vector.memset(zb, 0.0)
                    nc.sync.dma_start(csd, csp[:])
                    nc.sync.dma_start(tri, trip[:])
                    nc.sync.dma_start(ginv, ginvp[:])

            def emit_rope(ps_h, dst, hh, tab, on_dve=False):
                # psum rows 0:64 = x0 (even pair elems), 64:128 = x1.
                # dst[0:64] = x0*cos - x1*sin ; dst[64:128] = x0*sin + x1*cos
                # (tables carry the 1/A1 psum descale in fp8 mode)
                csA = csd[:, tab, :]
                csB = csd[:, tab + 1, :]
                sl = slice(hh * QB, (hh + 1) * QB)
                rc = rpool.tile([128, QB], MD, tag="rc", bufs=2)
                if on_dve:
                    # keep the scalar engine free for attention exps near
                    # the phase transition
                    nc.vector.tensor_scalar_add(rc, ps_h, 0.0)
                else:
                    nc.scalar.copy(rc, ps_h)    # frees the psum slot fast
                rs = rpool.tile([128, QB], MD, tag="rs", bufs=2)
                nc.sync.dma_start(rs[0:64, :], rc[64:128, :])
                nc.sync.dma_start(rs[64:128, :], rc[0:64, :])
                # tm1 = [x0*cos ; x1*cos], tm2 = [x1*sin ; x0*sin]
                tm1 = rpool.tile([128, QB], MD, tag="tm1", bufs=2)
                tm2 = rpool.tile([128, QB], MD, tag="tm2", bufs=2)
                nc.vector.tensor_mul(tm1, rc, csA[:, sl])
                nc.vector.tensor_mul(tm2, rs, csB[:, sl])
                nc.vector.tensor_sub(dst[0:64, sl], tm1[0:64, :], tm2[0:64, :])
                nc.vector.tensor_add(dst[64:128, sl], tm2[64:128, :],
                                     tm1[64:128, :])

            def emit_head_block(hs, b, WB):
                """One WB-chunk accumulation block of a K (hs=('k',j)) or
                Q (hs=('q',h,psq)) head.  hs[1] indexes the weight param;
                hs[-1] is the [psum tiles, pak?] state made at block 0."""
                kind = hs[0]
                psq, pak = hs[-1]
                wt = wpool.tile([128, NV, WB, HD], WD, tag="w",
                                name="wtk")
                src = wkp if kind == "k" else wqp
                nc.sync.dma_start(wt,
                                  src[hs[1], :, :, b * WB:(b + 1) * WB, :])
                for i in range(b * WB // XG, (b * WB + WB - 1) // XG + 1):
                    xload(i)
                xload(min(((b + 1) * WB) // XG, NX - 1))
                NBK = KO // WB
                if fp8:
                    for pi in range(WB // 2):
                        c = b * WB + 2 * pi
                        for ti, (xv, wv_) in enumerate(TERMS):
                            st = (b == 0 and pi == 0 and ti == 0)
                            sp = (b == NBK - 1 and pi == WB // 2 - 1
                                  and ti == len(TERMS) - 1)
                            wpr = wt[:, wv_, 2 * pi:2 * pi + 2, :]
                            for hh in range(NQH):
                                sl = slice(hh * QB, (hh + 1) * QB)
                                nc.tensor.matmul(
                                    psq[hh][:, :], wpr, xpair(c, xv, sl),
                                    start=st, stop=sp, perf_mode=DR)
                            if pak is not None:
                                nc.tensor.matmul(
                                    pak[:, :], wpr, adT[:, xv, c:c + 2, :],
                                    start=st, stop=sp, perf_mode=DR)
                else:
                    for ci in range(WB):
                        c = b * WB + ci
                        st, sp = (c == 0), (c == KO - 1)
                        for hh in range(NQH):
                            sl = slice(hh * QB, (hh + 1) * QB)
                            nc.tensor.matmul(
                                psq[hh][:, :], wt[0][:, ci, :], xsl(c)[:, sl],
                                start=st, stop=sp)
                        if pak is not None:
                            nc.tensor.matmul(
                                pak[:, :], wt[0][:, ci, :], adT[0][:, c, :],
                                start=st, stop=sp)

            def emit_head_finish(hs, on_dve=False):
                kind = hs[0]
                psq, pak = hs[-1]
                dst = kT[hs[1]] if kind == "k" else qT[hs[1]]
                tab = 2 if kind == "k" else 0
                for hh in range(NQH):
                    emit_rope(psq[hh], dst, hh, tab, on_dve)
                if pak is not None:
                    if fp8:
                        nc.scalar.mul(dst[:, S:SA], pak[:, 0:A], 1.0 / A1K)
                    else:
                        nc.scalar.copy(dst[:, S:SA], pak[:, 0:A])

            pakk = ps.tile([128, KVL * AP2], F32, tag="ak", name="pakk")

            def head_state(kind, idx):
                psq = [ps.tile([128, QB], F32, tag="proj", bufs=4,
                               name=f"ps_{kind}{idx}_{hh}")
                       for hh in range(NQH)]
                pak = pakk[:, idx * AP2:(idx + 1) * AP2] \
                    if kind == "k" else None
                return (kind, idx, (psq, pak))

            # Interleave the first K head's accumulation blocks (4 chunks
            # each) between V blocks so x tiles stream in exactly one
            # V-block ahead of every consumer, then run the remaining
            # heads with the full 8-chunk block size.
            emit_vblock(0)
            emit_vblock(1)
            k0 = head_state("k", 0)
            for b in range(KO // WBV):
                emit_head_block(k0, b, WBV)
                if b + 2 < NVB:
                    emit_vblock(b + 2)
            emit_head_finish(k0)
            WBQ = min(8, KO)
            deferred_rope = None
            for kind, idx in ([("k", j) for j in range(1, KVL)] +
                              [("q", h) for h in range(HL)]):
                hs = head_state(kind, idx)
                for b in range(KO // WBQ):
                    emit_head_block(hs, b, WBQ)
                if kind == "q" and idx == HL - 1 and HL > 1:
                    deferred_rope = hs   # finished after the first
                else:                    # attention prologue below
                    emit_head_finish(hs, on_dve=(kind == "q" and
                                                 idx >= HL - 2))

            # general (non-causal) mask tiles
            if not causal:
                mtt = [persist.tile([128, KC // 2, S], F32, tag=f"mt{i}",
                                    name=f"mt{i}")
                       for i in range(2)]
                nc.sync.dma_start(mtt[0], mtp[:, 0:KC // 2, :])
                nc.sync.dma_start(mtt[1], mtp[:, KC // 2:KC, :])

                def mtsl(kc):
                    return mtt[kc // (KC // 2)][:, kc % (KC // 2), :]

            # ones-column (value UDT in fp8 mode) / identity / -1e9 triangle
            ones_col = tri[:, 0, 127:128]
            eye = tri[:, 1, :]
            mneg = tri[:, 2, :]

            # ---------------- phase 2: attention --------------------------
            # Each unit's adapter PV + normalization tail is deferred
            # until after the NEXT unit's adapter scores + first three
            # score chunks, so the scalar engine is already working on
            # the next unit's exps while the PE drains the current tail.
            # one persistent denominator bank: dt rows alternate between
            # partition 0 and 64 per unit (subtile WAR reaches two units
            # back), da at 32, adapter scores at 96:106
            dtda = ps.tile([128, QB], F32, tag="ak", name="dtda")

            def emit_attn_prologue(h, qh):
                j = h // nrep
                qs, qe = qh * QB, (qh + 1) * QB
                if causal:
                    kcs = [kc for kc in range(KC) if kc * 128 < qe]
                else:
                    kcs = list(range(KC))
                # chunk widths; the last two chunks share one psum tile
                # and one exp instruction when they fit in a bank
                Ns = [qe - (max(qs, kc * 128) if causal else qs)
                      for kc in kcs]
                groups = [[ki] for ki in range(len(kcs))]
                if causal and len(kcs) >= 2 and Ns[-2] + Ns[-1] <= QB:
                    groups = groups[:-2] + [[len(kcs) - 2, len(kcs) - 1]]
                gof = {ki: gi for gi, g in enumerate(groups) for ki in g}
                st = {"h": h, "qh": qh, "j": j, "qs": qs, "qe": qe,
                      "kcs": kcs, "pts": {}, "groups": groups, "gof": gof}

                def emit_group(gi):
                    scp = ps.tile([128, QB], F32, tag="proj", bufs=4)
                    pt = spool.tile([128, QB], MD, tag="pt", bufs=4)
                    c0 = 0
                    for ki in groups[gi]:
                        kc = kcs[ki]
                        q0 = max(qs, kc * 128) if causal else qs
                        N = qe - q0
                        diag = causal and kc * 128 >= qs
                        nc.tensor.matmul(
                            scp[:, c0:c0 + N],
                            kT[j][:, kc * 128:(kc + 1) * 128],
                            qT[h][:, q0:qe], start=True, stop=not diag)
                        if diag:  # add -1e9 above the diagonal in psum
                            nc.tensor.matmul(scp[:, c0:c0 + 128], eye,
                                             mneg, start=False, stop=True)
                        if not causal:
                            sadd = spool.tile([128, QB], F32, tag="sadd",
                                              bufs=2)
                            nc.vector.tensor_add(
                                sadd[:, 0:N], scp[:, c0:c0 + N],
                                mtsl(kc)[:, q0:qe])
                            nc.scalar.activation(pt[:, c0:c0 + N],
                                                 sadd[:, 0:N], Exp,
                                                 bias=zb)
                        st["pts"][ki] = (pt, c0, q0, N)
                        c0 += N
                    if causal:
                        nc.scalar.activation(pt[:, 0:c0], scp[:, 0:c0],
                                             Exp, bias=zb)

                st["emit_group"] = emit_group
                st["next_g"] = min(3, len(groups))
                for gi in range(st["next_g"]):
                    emit_group(gi)
                # adapter scores after the token scores so their exp sits
                # behind the first token exps in the scalar-engine queue
                sca = ps.tile([128, QB], F32, tag="proj", bufs=4)
                nc.tensor.matmul(sca[0:A, :], kT[j][:, S:SA],
                                 qT[h][:, qs:qe], start=True, stop=True)
                pa = spool.tile([128, QB], MD, tag="pa", bufs=2)
                nc.scalar.activation(pa[0:A, :], sca[0:A, :], Exp,
                                     bias=zb[0:A, :])
                st["pa"] = pa
                return st

            def emit_attn_body(st, uidx):
                h, j, qs, kcs = st["h"], st["j"], st["qs"], st["kcs"]
                ot_ps = ps.tile([128, QB], F32, tag="vproj", bufs=2)
                oa_ps = ps.tile([128, QB], F32, tag="av", bufs=1)
                dr = 64 * (uidx % 2)
                st["ot_ps"], st["oa_ps"], st["dr"] = ot_ps, oa_ps, dr
                groups, gof = st["groups"], st["gof"]
                for ki, kc in enumerate(kcs):
                    pt, c0, q0, N = st["pts"].pop(ki)
                    s0, sp = (ki == 0), (ki == len(kcs) - 1)
                    nc.tensor.matmul(
                        ot_ps[:, q0 - qs:QB],
                        vv[:, kc, j * HD:(j + 1) * HD],
                        pt[:, c0:c0 + N], start=s0, stop=sp)
                    nc.tensor.matmul(
                        dtda[dr:dr + 1, q0 - qs:QB], ones_col[:, 0:1],
                        pt[:, c0:c0 + N], start=s0, stop=sp)
                    gi = gof[ki]
                    if ki == groups[gi][-1]:
                        # sca's slot frees after its exp, so the ring
                        # sustains four outstanding groups mid-unit
                        while (st["next_g"] < len(groups) and
                               st["next_g"] <= gi + 4):
                            st["emit_group"](st["next_g"])
                            st["next_g"] += 1
                # gated adapter denominator (UDT/tanh folded into the
                # contraction column)
                nc.tensor.matmul(dtda[32:33, :], ginv[0:A, h:h + 1],
                                 st["pa"][0:A, :], start=True, stop=True)
                rt = spool.tile([1, QB], F32, tag="rt", bufs=2)
                nc.vector.reciprocal(rt, dtda[dr:dr + 1, :])
                ra = spool.tile([1, QB], F32, tag="ra", bufs=2)
                nc.vector.reciprocal(ra, dtda[32:33, :])
                rtb = spool.tile([128, QB], F32, tag="rtb", bufs=2)
                nc.gpsimd.partition_broadcast(rtb, rt[0:1, :])
                rab = spool.tile([128, QB], F32, tag="rab", bufs=2)
                nc.gpsimd.partition_broadcast(rab, ra[0:1, :])
                st["rtb"], st["rab"] = rtb, rab

            def emit_attn_tail(st):
                h, qh, j = st["h"], st["qh"], st["j"]
                qs, qe = st["qs"], st["qe"]
                pa, ot_ps, oa_ps = st["pa"], st["ot_ps"], st["oa_ps"]
                nc.tensor.matmul(oa_ps[:, :],
                                 vv[0:A, KC, j * HD:(j + 1) * HD],
                                 pa[0:A, :], start=True, stop=True)
                # oT = ot/denom_t + tanh(g)*oa/denom_a  (write-once; the
                # fp8 path emits GAMO-scaled hi/lo e4m3 for the O proj)
                tq1 = spool.tile([128, QB], F32, tag="tq1", bufs=1)
                nc.vector.tensor_mul(tq1, ot_ps[:, :], st["rtb"])
                tq2 = spool.tile([128, QB], F32, tag="tq2", bufs=1)
                nc.vector.tensor_mul(tq2, oa_ps[:, :], st["rab"])
                if fp8:
                    tsum = spool.tile([128, QB], F32, tag="tsum", bufs=1)
                    nc.vector.tensor_add(tsum, tq1, tq2)
                    nc.scalar.copy(oT8[:, 0, h, qs:qe], tsum)
                    nc.vector.tensor_sub(oT8[:, 1, h, qs:qe], tsum,
                                         oT8[:, 0, h, qs:qe])
                else:
                    nc.vector.tensor_add(oT[h][:, qs:qe], tq1, tq2)

            units = [(h, qh) for h in range(HL) for qh in range(NQH)]
            pending = None
            for uidx, (h, qh) in enumerate(units):
                st = emit_attn_prologue(h, qh)
                if deferred_rope is not None:
                    emit_head_finish(deferred_rope, on_dve=True)
                    deferred_rope = None
                if pending is not None:
                    emit_attn_tail(pending)
                emit_attn_body(st, uidx)
                pending = st
            emit_attn_tail(pending)

            # ---------------- phase 3: output projection ------------------
            # wo weights stream through a ring, DMA'd ahead of
            # consumption.  The last output tile is split into halves to
            # shorten the drain tail.
            won = {}

            def load_won(n):
                if n < NB:
                    wt = wpool.tile([128, NV, HL, 512], WD, tag="won",
                                    bufs=4, name=f"won{n}")
                    won[n] = wt
                    nc.sync.dma_start(
                        wt, wop[:, :, :, n * 512:(n + 1) * 512])

            for n in range(min(4, NB)):
                load_won(n)
            for n in range(NB):
                wt = won.pop(n)
                for m in range(NM):
                    last = (n == NB - 1) and (m == NM - 1)
                    halves = ((0, 256), (256, 512)) if last else ((0, 512),)
                    for c0, c1 in halves:
                        pso = ps.tile([128, 512], F32, tag="proj", bufs=4)
                        msl = slice(m * 128, (m + 1) * 128)
                        if fp8:
                            n_ = 0
                            NT = len(TERMS) * (HL // 2)
                            for ov, wv_ in TERMS:
                                for hp in range(HL // 2):
                                    nc.tensor.matmul(
                                        pso[:, 0:c1 - c0],
                                        oT8[:, ov, 2 * hp:2 * hp + 2, msl],
                                        wt[:, wv_, 2 * hp:2 * hp + 2, c0:c1],
                                        start=(n_ == 0), stop=(n_ == NT - 1),
                                        perf_mode=DR)
                                    n_ += 1
                        else:
                            for hh in range(HL):
                                nc.tensor.matmul(
                                    pso[:, 0:c1 - c0],
                                    oT[hh][:, msl],
                                    wt[:, 0, hh, c0:c1],
                                    start=(hh == 0), stop=(hh == HL - 1))
                        ob = obpool.tile([128, 512], MD if fp8 else F32,
                                         tag="ob")
                        if fp8:
                            nc.scalar.mul(ob[:, 0:c1 - c0],
                                          pso[:, 0:c1 - c0], OSC)
                        else:
                            nc.scalar.copy(ob[:, 0:c1 - c0],
                                           pso[:, 0:c1 - c0])
                        nc.sync.dma_start(
                            outp[m, :, n * 512 + c0:n * 512 + c1],
                            ob[:, 0:c1 - c0])
                load_won(n + 4)

    nc.compile()
    nc.finalize()
    return nc


def get_program(KO, S, HL, KVL, causal, mm):
    key = (KO, S, HL, KVL, causal, mm)
    if key not in _PROG_CACHE:
        _PROG_CACHE[key] = build_program(KO, S, HL, KVL, causal, mm)
    return _PROG_CACHE[key]


# --------------------------------------------------------------------------
# host-side sharding / layout prep
# --------------------------------------------------------------------------

_EVEN_FIRST = np.concatenate([np.arange(0, HD, 2), np.arange(1, HD, 2)])


def is_causal_mask(mask):
    S = mask.shape[-1]
    m = np.asarray(mask).reshape(S, S)
    iu = np.triu_indices(S, 1)
    il = np.tril_indices(S)
    return bool(np.all(m[il] == 0.0) and np.all(m[iu] <= -1e8))


def _np_md(mm):
    if mm in ("bf16", "fp8"):
        import ml_dtypes
        return ml_dtypes.bfloat16
    return np.float32


def _np_f8():
    import ml_dtypes
    return ml_dtypes.float8_e4m3


def _q8pair(a32):
    """Split scaled f32 array into e4m3 hi + exact-residual lo."""
    f8 = _np_f8()
    h = np.clip(a32, -240.0, 240.0).astype(f8)
    l = np.clip(a32 - h.astype(np.float32), -240.0, 240.0).astype(f8)
    return h, l


def prep_core_inputs(core, G, x, wq, wk, wv, wo, adapter, gate,
                     freqs_cos, freqs_sin, mask, causal, mm=None):
    """Build the input dict for one core = (batch b, head-group g)."""
    mm = MM_MODE if mm is None else mm
    fp8 = mm == "fp8"
    B, S, D = x.shape
    H = gate.shape[1]
    hd = wq.shape[1] // H
    KV = wk.shape[1] // hd
    KO = D // 128
    KC = S // 128
    HL, KVL = H // G, KV // G
    b, g = core // G, core % G
    hsl = slice(g * HL, (g + 1) * HL)
    ksl = slice(g * KVL, (g + 1) * KVL)
    idx = _EVEN_FIRST
    f32 = np.float32
    md = _np_md(mm)

    def c(a, dt=None):
        return np.ascontiguousarray(a, dtype=dt if dt is not None else md)

    def pairs(key, a32, axis):
        if fp8:
            h, l = _q8pair(np.ascontiguousarray(a32, f32))
            return {key: np.ascontiguousarray(np.stack([h, l], axis=axis))}
        return {key: c(np.expand_dims(a32, axis))}

    inp = {}
    xp = x[b].T.reshape(KO, 128, S).transpose(1, 0, 2)
    inp.update(pairs("xp", xp * AX if fp8 else xp, 1))
    wq4 = wq.reshape(D, H, hd)[:, hsl][:, :, idx] * np.float32(1.0 / np.sqrt(hd))
    wq4 = wq4.reshape(KO, 128, HL, hd).transpose(2, 1, 0, 3)
    inp.update(pairs("wqp", wq4 * AWQ if fp8 else wq4, 2))
    wk4 = wk.reshape(D, KV, hd)[:, ksl][:, :, idx]
    wk4 = wk4.reshape(KO, 128, KVL, hd).transpose(2, 1, 0, 3)
    inp.update(pairs("wkp", wk4 * AWK if fp8 else wk4, 2))
    wv4 = wv.reshape(D, KV, hd)[:, ksl]
    wv4 = wv4.reshape(KO, 128, KVL * hd).transpose(1, 0, 2)
    inp.update(pairs("wvp", wv4 * AWV if fp8 else wv4, 1))
    wos = wo[g * HL * hd:(g + 1) * HL * hd]
    wos = wos.reshape(HL, hd, D).transpose(1, 0, 2)
    inp.update(pairs("wop", wos * AWO if fp8 else wos, 1))
    adx = adapter[0].T.reshape(KO, 128, A).transpose(1, 0, 2)
    adxp = np.zeros((128, KO, AP2), f32)
    adxp[:, :, :A] = adx
    inp.update(pairs("adp", adxp * AX if fp8 else adxp, 1))
    # cos^T / sin^T, duplicated across both partition halves; q rows 0-1
    # carry the q-psum descale, k rows 2-3 the k-psum descale (fp8 mode)
    ct = np.asarray(freqs_cos, dtype=f32).T      # [64, S]
    st = np.asarray(freqs_sin, dtype=f32).T
    sq = 1.0 / A1Q if fp8 else 1.0
    sk = 1.0 / A1K if fp8 else 1.0
    csp = np.empty((128, 4, S), f32)
    for half in (slice(0, 64), slice(64, 128)):
        csp[half, 0] = ct * sq
        csp[half, 1] = st * sq
        csp[half, 2] = ct * sk
        csp[half, 3] = st * sk
    csp = c(csp)
    triu = np.triu(np.ones((128, 128), dtype=f32))
    tri = np.empty((128, 3, 128), f32)
    tri[:, 0] = (UDT if fp8 else 1.0) * np.ones((128, 128), f32)
    tri[:, 1] = np.eye(128, dtype=f32)
    tri[:, 2] = -1e9 * (1.0 - triu)
    tri = c(tri)
    gth = np.tanh(np.asarray(gate[0, hsl, 0, 0], dtype=np.float64)).astype(f32)
    with np.errstate(divide="ignore"):
        gi = (UDT if fp8 else 1.0) / gth
        ginv = np.broadcast_to(gi.reshape(1, HL), (128, HL))
    ginvp = c(ginv)
    inp.update({"csp": csp, "trip": tri, "ginvp": ginvp})
    if not causal:
        mt = np.asarray(mask).reshape(S, S).T  # [keys, q]
        inp["mtp"] = c(mt.reshape(KC, 128, S).transpose(1, 0, 2), f32)
    return inp


# --------------------------------------------------------------------------
# entry point
# --------------------------------------------------------------------------

def kernel(x, wq, wk, wv, wo, adapter, gate, freqs_cos, freqs_sin, mask,
           _trace=False):
    x, wq, wk, wv, wo, adapter, gate, freqs_cos, freqs_sin, mask = (
        np.asarray(a) for a in
        (x, wq, wk, wv, wo, adapter, gate, freqs_cos, freqs_sin, mask))
    B, S, D = x.shape
    H = gate.shape[1]
    hd = wq.shape[1] // H
    KV = wk.shape[1] // hd
    G = 8 // B                      # head groups per batch over 8 cores
    HL, KVL = H // G, KV // G
    KO = D // 128

    causal = is_causal_mask(mask)
    nc = get_program(KO, S, HL, KVL, causal, MM_MODE)

    in_maps = [prep_core_inputs(core, G, x, wq, wk, wv, wo, adapter, gate,
                                freqs_cos, freqs_sin, mask, causal)
               for core in range(8)]
    res = run_bass_kernel_spmd(nc, in_maps, core_ids=list(range(8)),
                               trace=_trace)
    out = np.zeros((B, S, D), np.float32)
    for core in range(8):
        b = core // G
        r = res.results[core]
        out[b] += np.asarray(r["out"], np.float32).reshape(S, D)
    if _trace:
        kernel._last_result = res
    return out


# revision 41
# speedup vs baseline: 1.1935x; 1.0049x over previous
"""Trainium2 Bass kernel for nn_Attention_50216757625003.

GQA attention layer: B=2, S=1024, D=4096, H=32 q-heads, KV=8 kv-heads,
hd=128, A=10 gated adapter tokens, RoPE, split softmax (adapter block
softmaxed separately and scaled by tanh(gate)), causal mask.

Sharding (8 NeuronCores): outer data-parallel over batch (2) x
tensor-parallel over heads (4 groups of 8 q-heads / 2 kv-heads).
wq/wk/wv are sharded column-wise, wo row-wise; each core computes a
partial [S, D] output contribution and the host sums the 4 head-group
partials per batch element.

Device-side layout tricks:
  * x is fed transposed ([D, S]) so all projections run with D on the
    contraction (partition) axis.
  * q/k head dims are permuted even-first on the host (wq/wk column
    permutation); RoPE pairs then live on partitions p and p+64.  A
    cheap SBUF->SBUF DMA swaps the halves so the rotation becomes four
    partition-aligned DVE ops against duplicated cos/sin tables.
  * scores are built transposed ([keys, q]) so softmax denominators come
    from a ones-vector matmul and probs feed the PV matmul directly (no
    transposes anywhere).
  * softmax skips the max-subtraction (scores are O(1) here; exp is safe
    in fp32), which the per-block normalization keeps exact.
  * default KMM=fp8 mode: the four big projections (Q/K/V/O) run as
    fp8e4 DoubleRow matmuls (0.5 cycles/row, two 128-deep k-tiles per
    instruction).  Every projection operand is split hi/lo into two
    e4m3 tensors sharing one power-of-2 scale (lo = exact residual of
    hi), and each product is evaluated with three terms
    (hi*hi + lo*hi + hi*lo), which restores bf16-level accuracy at
    0.75x the bf16 cycle count.  x / weights / adapter splits are
    precomputed on the host; the attention output is split on the fly
    (one extra scalar copy + DVE subtract per tile).  All scale
    factors fold into host-prepped constants: the RoPE tables absorb
    the Q/K psum descale (4-row table: q rows then k rows), the
    softmax-denominator ones-column and the gate column absorb the
    V-path scale, and the output copy descales by a single immediate.
    Attention itself (scores, exp, PV) stays bf16: the score
    contraction is only hd=128 so DoubleRow pairing would cost 1.5x,
    and fp8 probs would need an extra elementwise residual pass that
    the scalar engine can't absorb.  The adapter dim is zero-padded
    10->16 so DoubleRow pair strides stay 16-byte aligned, both kv
    heads' adapter-K psums share one bank (disjoint column ranges),
    and output partials leave the core as bf16 (the host sums in
    fp32), halving output DMA in the DMA-tight O-projection phase.
  * ONE psum pool spans all phases (tags: proj/vproj/av/ak, 8 banks
    exactly); avoiding pool close/reopen kills the cross-engine barrier
    stalls between projection, attention, and output phases.
  * the first K head's accumulation runs in 4-chunk blocks interleaved
    between V blocks, so x tiles stream in exactly one V-block ahead of
    every consumer and the PE never starves on the x DMA burst.
  * attention emits scores three chunk-groups ahead of the PV matmuls
    (each unit's last two key chunks share one psum tile and a single
    exp) and defers each unit's adapter-PV/normalization tail until
    after the next unit's prologue, hiding the scalar-engine exp
    latency and instruction overhead.
  * the causal diagonal is masked by a PE psum-accumulate of an
    additive -1e9 tile (identity lhsT), not a DVE multiply after exp.
  * softmax denominators live in one persistent psum bank (dt rows
    alternate partitions 0/64, gated adapter denominator at 32 with
    the V-scale/tanh(gate) factors folded into its contraction
    column); their reciprocals are partition-broadcast on the
    otherwise-idle GpSimd engine.
  * RoPE runs in bf16 (2x DVE) off the psum via an SBUF half-swap DMA;
    the psum-free copy moves to the DVE for the last two q-heads to
    keep the scalar engine clear for the first attention exps.
  * wo weights stream through a ring DMA'd during attention; the last
    output tile is split to shorten the drain tail; the last q-head's
    RoPE is deferred past the first attention prologue.
"""

import os
import sys

import numpy as np

for _p in ("/opt/trn_rl_repo",):
    if _p not in sys.path and os.path.isdir(_p):
        sys.path.insert(0, _p)

import concourse.bass as bass
import concourse.mybir as mybir
from concourse import bacc
import concourse.tile as tile
from concourse.bass_utils import run_bass_kernel_spmd

HD = 128  # head dim (hardcoded: rope split + tile shapes assume 128)
A = 10    # adapter tokens
AP2 = 16  # adapter dim padded so the DoubleRow pair step is 16-aligned
F32 = mybir.dt.float32
F8 = mybir.dt.float8e4

MM_MODE = os.environ.get("KMM", "fp8")

# fp8 power-of-2 scale plan (see module docstring):
#   x, adapter-x:    * 2^4      (sigma 16, max ~84 < 240)
#   wq (with 1/sqrt(hd) folded): * 2^14  (sigma ~22, max ~118)
#   wk, wv, wo:      * 2^10     (sigma 16, max ~83)
#   oT (runtime):    * 2^5      (|oT| <= ~5 -> max ~160)
AX = 2.0 ** 4
AWQ = 2.0 ** 14
AWK = 2.0 ** 10
AWV = 2.0 ** 10
AWO = 2.0 ** 10
GAMO = 2.0 ** 5
A1Q = AX * AWQ          # q psum scale
A1K = AX * AWK          # k psum scale
A1V = AX * AWV          # v path scale
UDT = A1V / GAMO        # ones-column value: dt_psum = UDT * sum(exp)
OSC = 1.0 / (GAMO * AWO)  # final output copy scale

_PROG_CACHE = {}


def _md(mm):
    return {"f32r": mybir.dt.float32r, "f32": mybir.dt.float32,
            "bf16": mybir.dt.bfloat16, "fp8": mybir.dt.bfloat16}[mm]


# --------------------------------------------------------------------------
# device program
# --------------------------------------------------------------------------

def build_program(KO, S, HL, KVL, causal, mm):
    """One NeuronCore's program.

    KO: D // 128 contraction chunks.  S: sequence length.  HL: q heads on
    this core.  KVL: kv heads on this core.  causal: hardwire causal
    masking (tri mask on diagonal chunks + chunk skipping); otherwise an
    additive mask [S, S] is an input.  mm: matmul operand dtype mode
    ("fp8" = DoubleRow fp8 projections + bf16 attention).
    """
    nc = bacc.Bacc(None, target_bir_lowering=False,
                   dynamic_dma_scratch_size=2048)
    MD = _md(mm)          # attention operand dtype (bf16 in fp8 mode)
    fp8 = mm == "fp8"
    WD = F8 if fp8 else MD  # projection operand dtype
    DR = mybir.MatmulPerfMode.DoubleRow if fp8 else None
    NV = 2 if fp8 else 1  # hi/lo variants per projection operand
    # product terms (x variant, w variant): hi*hi, lo*hi, hi*lo
    TERMS = ((0, 0), (1, 0), (0, 1)) if fp8 else ((0, 0),)
    D = KO * 128
    QB = min(512, S)       # q column block (psum bank + fp32 moving max)
    NQH = S // QB
    KC = S // 128          # token key chunks
    SA = S + A
    nrep = HL // KVL
    NB = D // 512          # wo column blocks
    NM = S // 128          # wo row chunks

    def dparam(name, shape, dt_):
        return nc.declare_dram_parameter(name, shape, dt_, isOutput=False)

    # hi/lo variants packed on one axis right after the partition dim:
    # one DMA per block loads both, and the innermost runs stay >= 512 B
    xp = dparam("xp", [128, NV, KO, S], WD)
    wqp = dparam("wqp", [HL, 128, NV, KO, HD], WD)
    wkp = dparam("wkp", [KVL, 128, NV, KO, HD], WD)
    wvp = dparam("wvp", [128, NV, KO, KVL * HD], WD)
    wop = dparam("wop", [128, NV, HL, D], WD)
    adp = dparam("adp", [128, NV, KO, AP2], WD)
    csp = dparam("csp", [128, 4, S], MD)
    trip = dparam("trip", [128, 3, 128], MD)
    ginvp = dparam("ginvp", [128, HL], MD)
    if not causal:
        mtp = dparam("mtp", [128, KC, S], F32)
    outp = nc.declare_dram_parameter("out", [NM, 128, D],
                                     MD if fp8 else F32, isOutput=True)

    Exp = mybir.ActivationFunctionType.Exp

    with tile.TileContext(nc) as tc:
        with tc.tile_pool(name="persist", bufs=1) as persist, \
             tc.tile_pool(name="wpool", bufs=4) as wpool, \
             tc.tile_pool(name="rpool", bufs=2) as rpool, \
             tc.tile_pool(name="cpool", bufs=1) as cpool, \
             tc.tile_pool(name="spool", bufs=1) as spool, \
             tc.tile_pool(name="obpool", bufs=4) as obpool, \
             tc.tile_pool(name="ps", bufs=1, space="PSUM") as ps:

            # resident x^T in XG-chunk tiles, DMA'd just-in-time from the
            # V-projection loop so the first matmuls start early
            XG = min(4, KO)
            NX = KO // XG
            xt = [persist.tile([128, NV, XG, S], WD, tag=f"x_{i}",
                               name=f"x_{i}")
                  for i in range(NX)]
            xt_loaded = [False] * NX

            def xload(i):
                if not xt_loaded[i]:
                    if i == 0:
                        # first hi k-pair alone so the first matmul can
                        # start as soon as one k-pair + one weight block land
                        c1 = min(2, XG)
                        nc.sync.dma_start(xt[i][:, 0:1, 0:c1, :],
                                          xp[:, 0:1, 0:c1, :])
                        if XG > c1:
                            nc.sync.dma_start(xt[i][:, 0:1, c1:XG, :],
                                              xp[:, 0:1, c1:XG, :])
                        if NV > 1:
                            nc.sync.dma_start(xt[i][:, 1:NV, :, :],
                                              xp[:, 1:NV, 0:XG, :])
                    else:
                        nc.sync.dma_start(
                            xt[i], xp[:, :, i * XG:(i + 1) * XG, :])
                    xt_loaded[i] = True

            def xpair(c, v, sl):
                # [128, 2, sl] k-tile pair starting at chunk c (c even-offset)
                t_ = xt[c // XG]
                cc = c % XG
                return t_[:, v, cc:cc + 2, sl]

            def xsl(c):
                return xt[c // XG][:, 0, c % XG, :]

            kT = [persist.tile([128, SA], MD, tag=f"kT{j}", name=f"kT{j}")
                  for j in range(KVL)]
            vv = persist.tile([128, KC + 1, KVL * HD], MD, tag="vv")
            qT = [persist.tile([128, S], MD, tag=f"qT{h}", name=f"qT{h}")
                  for h in range(HL)]
            if fp8:
                # attention output hi/lo fp8 (head dim packed for DoubleRow
                # head-pairing in the O projection)
                oT8 = persist.tile([128, 2, HL, S], F8, tag="oT8")
            else:
                oT = [persist.tile([128, S], MD, tag=f"oT{h}", name=f"oT{h}")
                      for h in range(HL)]

            # cos/sin tables (q rows 0-1, k rows 2-3), adapter x^T,
            # folded gate column, tri mask
            csd = cpool.tile([128, 4, S], MD)
            adT = cpool.tile([128, NV, KO, AP2], WD)
            ginv = cpool.tile([128, HL], MD)
            tri = cpool.tile([128, 3, 128], MD)
            vacc = cpool.tile([128, KC, KVL * HD], F32)
            zb = cpool.tile([128, 1], F32)

            # ---------------- phase 1: projections -----------------------
            pav = ps.tile([AP2, KVL * HD], F32, tag="av")

            WBV = min(4, KO)
            NVB = KO // WBV

            def emit_vblock(b):
                wt = wpool.tile([128, NV, WBV, KVL * HD], WD, tag="w",
                                name="wtv")
                nc.sync.dma_start(wt, wvp[:, :, b * WBV:(b + 1) * WBV, :])
                for i in range(b * WBV // XG,
                               (b * WBV + WBV - 1) // XG + 1):
                    xload(i)
                if b == 0:
                    nc.sync.dma_start(adT, adp[:])
                for t in range(KC):
                    psv = ps.tile([128, KVL * HD], F32, tag="vproj",
                                  bufs=2)
                    tsl = slice(t * 128, (t + 1) * 128)
                    if fp8:
                        n = 0
                        NT = len(TERMS) * (WBV // 2)
                        for xv, wv_ in TERMS:
                            for pi in range(WBV // 2):
                                c = b * WBV + 2 * pi
                                nc.tensor.matmul(
                                    psv[:, :], xpair(c, xv, tsl),
                                    wt[:, wv_, 2 * pi:2 * pi + 2, :],
                                    start=(n == 0), stop=(n == NT - 1),
                                    perf_mode=DR)
                                n += 1
                    else:
                        for ci in range(WBV):
                            c = b * WBV + ci
                            nc.tensor.matmul(
                                psv[:, :], xsl(c)[:, tsl], wt[:, 0, ci, :],
                                start=(ci == 0), stop=(ci == WBV - 1))
                    if b == 0 and NVB > 1:
                        nc.scalar.copy(vacc[:, t, :], psv[:, :])
                    elif b < NVB - 1:
                        nc.vector.tensor_add(vacc[:, t, :], vacc[:, t, :],
                                             psv[:, :])
                    elif NVB > 1:
                        nc.vector.tensor_add(vv[:, t, :], vacc[:, t, :],
                                             psv[:, :])
                    else:
                        nc.scalar.copy(vv[:, t, :], psv[:, :])
                if fp8:
                    for ti, (xv, wv_) in enumerate(TERMS):
                        for pi in range(WBV // 2):
                            c = b * WBV + 2 * pi
                            nc.tensor.matmul(
                                pav[:, :], adT[:, xv, c:c + 2, :],
                                wt[:, wv_, 2 * pi:2 * pi + 2, :],
                                start=(b == 0 and ti == 0 and pi == 0),
                                stop=(b == NVB - 1 and ti == len(TERMS) - 1
                                      and pi == WBV // 2 - 1),
                                perf_mode=DR)
                else:
                    for ci in range(WBV):
                        c = b * WBV + ci
                        nc.tensor.matmul(pav[0:A, :], adT[:, 0, c, 0:A],
                                         wt[:, 0, ci, :],
                                         start=(c == 0), stop=(c == KO - 1))
                if b == NVB - 1:
                    nc.scalar.copy(vv[0:A, KC, :], pav[0:A, :])
                if b == 0:
                    nc.vector.memset(zb, 0.0)
                    nc.sync.dma_start(tri, trip[:])
                    nc.sync.dma_start(ginv, ginvp[:])
                if b == 2:
                    # 1 MB table, first needed by the k0 rope (~35us):
                    # keep it out of the cold-start DMA crunch
                    nc.sync.dma_start(csd, csp[:])

            def emit_rope(ps_h, dst, hh, tab, on_dve=False):
                # psum rows 0:64 = x0 (even pair elems), 64:128 = x1.
                # dst[0:64] = x0*cos - x1*sin ; dst[64:128] = x0*sin + x1*cos
                # (tables carry the 1/A1 psum descale in fp8 mode)
                csA = csd[:, tab, :]
                csB = csd[:, tab + 1, :]
                sl = slice(hh * QB, (hh + 1) * QB)
                rc = rpool.tile([128, QB], MD, tag="rc", bufs=2)
                if on_dve:
                    # keep the scalar engine free for attention exps near
                    # the phase transition
                    nc.vector.tensor_scalar_add(rc, ps_h, 0.0)
                else:
                    nc.scalar.copy(rc, ps_h)    # frees the psum slot fast
                rs = rpool.tile([128, QB], MD, tag="rs", bufs=2)
                nc.sync.dma_start(rs[0:64, :], rc[64:128, :])
                nc.sync.dma_start(rs[64:128, :], rc[0:64, :])
                # tm1 = [x0*cos ; x1*cos], tm2 = [x1*sin ; x0*sin]
                tm1 = rpool.tile([128, QB], MD, tag="tm1", bufs=2)
                tm2 = rpool.tile([128, QB], MD, tag="tm2", bufs=2)
                nc.vector.tensor_mul(tm1, rc, csA[:, sl])
                nc.vector.tensor_mul(tm2, rs, csB[:, sl])
                nc.vector.tensor_sub(dst[0:64, sl], tm1[0:64, :], tm2[0:64, :])
                nc.vector.tensor_add(dst[64:128, sl], tm2[64:128, :],
                                     tm1[64:128, :])

            def emit_head_block(hs, b, WB):
                """One WB-chunk accumulation block of a K (hs=('k',j)) or
                Q (hs=('q',h,psq)) head.  hs[1] indexes the weight param;
                hs[-1] is the [psum tiles, pak?] state made at block 0."""
                kind = hs[0]
                psq, pak = hs[-1]
                wt = wpool.tile([128, NV, WB, HD], WD, tag="w",
                                name="wtk")
                src = wkp if kind == "k" else wqp
                nc.sync.dma_start(wt,
                                  src[hs[1], :, :, b * WB:(b + 1) * WB, :])
                for i in range(b * WB // XG, (b * WB + WB - 1) // XG + 1):
                    xload(i)
                xload(min(((b + 1) * WB) // XG, NX - 1))
                NBK = KO // WB
                if fp8:
                    for pi in range(WB // 2):
                        c = b * WB + 2 * pi
                        for ti, (xv, wv_) in enumerate(TERMS):
                            st = (b == 0 and pi == 0 and ti == 0)
                            sp = (b == NBK - 1 and pi == WB // 2 - 1
                                  and ti == len(TERMS) - 1)
                            wpr = wt[:, wv_, 2 * pi:2 * pi + 2, :]
                            for hh in range(NQH):
                                sl = slice(hh * QB, (hh + 1) * QB)
                                nc.tensor.matmul(
                                    psq[hh][:, :], wpr, xpair(c, xv, sl),
                                    start=st, stop=sp, perf_mode=DR)
                            if pak is not None:
                                nc.tensor.matmul(
                                    pak[:, :], wpr, adT[:, xv, c:c + 2, :],
                                    start=st, stop=sp, perf_mode=DR)
                else:
                    for ci in range(WB):
                        c = b * WB + ci
                        st, sp = (c == 0), (c == KO - 1)
                        for hh in range(NQH):
                            sl = slice(hh * QB, (hh + 1) * QB)
                            nc.tensor.matmul(
                                psq[hh][:, :], wt[0][:, ci, :], xsl(c)[:, sl],
                                start=st, stop=sp)
                        if pak is not None:
                            nc.tensor.matmul(
                                pak[:, :], wt[0][:, ci, :], adT[0][:, c, :],
                                start=st, stop=sp)

            def emit_head_finish(hs, on_dve=False):
                kind = hs[0]
                psq, pak = hs[-1]
                dst = kT[hs[1]] if kind == "k" else qT[hs[1]]
                tab = 2 if kind == "k" else 0
                for hh in range(NQH):
                    emit_rope(psq[hh], dst, hh, tab, on_dve)
                if pak is not None:
                    if fp8:
                        nc.scalar.mul(dst[:, S:SA], pak[:, 0:A], 1.0 / A1K)
                    else:
                        nc.scalar.copy(dst[:, S:SA], pak[:, 0:A])

            pakk = ps.tile([128, KVL * AP2], F32, tag="ak", name="pakk")

            def head_state(kind, idx):
                psq = [ps.tile([128, QB], F32, tag="proj", bufs=4,
                               name=f"ps_{kind}{idx}_{hh}")
                       for hh in range(NQH)]
                pak = pakk[:, idx * AP2:(idx + 1) * AP2] \
                    if kind == "k" else None
                return (kind, idx, (psq, pak))

            # Interleave the first K head's accumulation blocks (4 chunks
            # each) between V blocks so x tiles stream in exactly one
            # V-block ahead of every consumer, then run the remaining
            # heads with the full 8-chunk block size.
            emit_vblock(0)
            emit_vblock(1)
            k0 = head_state("k", 0)
            for b in range(KO // WBV):
                emit_head_block(k0, b, WBV)
                if b + 2 < NVB:
                    emit_vblock(b + 2)
            emit_head_finish(k0)
            WBQ = min(8, KO)
            deferred_rope = None
            for kind, idx in ([("k", j) for j in range(1, KVL)] +
                              [("q", h) for h in range(HL)]):
                hs = head_state(kind, idx)
                for b in range(KO // WBQ):
                    emit_head_block(hs, b, WBQ)
                if kind == "q" and idx == HL - 1 and HL > 1:
                    deferred_rope = hs   # finished after the first
                else:                    # attention prologue below
                    emit_head_finish(hs, on_dve=(kind == "q" and
                                                 idx >= HL - 2))

            # general (non-causal) mask tiles
            if not causal:
                mtt = [persist.tile([128, KC // 2, S], F32, tag=f"mt{i}",
                                    name=f"mt{i}")
                       for i in range(2)]
                nc.sync.dma_start(mtt[0], mtp[:, 0:KC // 2, :])
                nc.sync.dma_start(mtt[1], mtp[:, KC // 2:KC, :])

                def mtsl(kc):
                    return mtt[kc // (KC // 2)][:, kc % (KC // 2), :]

            # ones-column (value UDT in fp8 mode) / identity / -1e9 triangle
            ones_col = tri[:, 0, 127:128]
            eye = tri[:, 1, :]
            mneg = tri[:, 2, :]

            # ---------------- phase 2: attention --------------------------
            # Each unit's adapter PV + normalization tail is deferred
            # until after the NEXT unit's adapter scores + first three
            # score chunks, so the scalar engine is already working on
            # the next unit's exps while the PE drains the current tail.
            # one persistent denominator bank: dt rows alternate between
            # partition 0 and 64 per unit (subtile WAR reaches two units
            # back), da at 32, adapter scores at 96:106
            dtda = ps.tile([128, QB], F32, tag="ak", name="dtda")

            def emit_attn_prologue(h, qh):
                j = h // nrep
                qs, qe = qh * QB, (qh + 1) * QB
                if causal:
                    kcs = [kc for kc in range(KC) if kc * 128 < qe]
                else:
                    kcs = list(range(KC))
                # chunk widths; the last two chunks share one psum tile
                # and one exp instruction when they fit in a bank
                Ns = [qe - (max(qs, kc * 128) if causal else qs)
                      for kc in kcs]
                groups = [[ki] for ki in range(len(kcs))]
                if causal and len(kcs) >= 2 and Ns[-2] + Ns[-1] <= QB:
                    groups = groups[:-2] + [[len(kcs) - 2, len(kcs) - 1]]
                gof = {ki: gi for gi, g in enumerate(groups) for ki in g}
                st = {"h": h, "qh": qh, "j": j, "qs": qs, "qe": qe,
                      "kcs": kcs, "pts": {}, "groups": groups, "gof": gof}

                def emit_group(gi):
                    scp = ps.tile([128, QB], F32, tag="proj", bufs=4)
                    pt = spool.tile([128, QB], MD, tag="pt", bufs=4)
                    c0 = 0
                    for ki in groups[gi]:
                        kc = kcs[ki]
                        q0 = max(qs, kc * 128) if causal else qs
                        N = qe - q0
                        diag = causal and kc * 128 >= qs
                        nc.tensor.matmul(
                            scp[:, c0:c0 + N],
                            kT[j][:, kc * 128:(kc + 1) * 128],
                            qT[h][:, q0:qe], start=True, stop=not diag)
                        if diag:  # add -1e9 above the diagonal in psum
                            nc.tensor.matmul(scp[:, c0:c0 + 128], eye,
                                             mneg, start=False, stop=True)
                        if not causal:
                            sadd = spool.tile([128, QB], F32, tag="sadd",
                                              bufs=2)
                            nc.vector.tensor_add(
                                sadd[:, 0:N], scp[:, c0:c0 + N],
                                mtsl(kc)[:, q0:qe])
                            nc.scalar.activation(pt[:, c0:c0 + N],
                                                 sadd[:, 0:N], Exp,
                                                 bias=zb)
                        st["pts"][ki] = (pt, c0, q0, N)
                        c0 += N
                    if causal:
                        nc.scalar.activation(pt[:, 0:c0], scp[:, 0:c0],
                                             Exp, bias=zb)

                st["emit_group"] = emit_group
                st["next_g"] = min(3, len(groups))
                for gi in range(st["next_g"]):
                    emit_group(gi)
                # adapter scores after the token scores so their exp sits
                # behind the first token exps in the scalar-engine queue
                sca = ps.tile([128, QB], F32, tag="proj", bufs=4)
                nc.tensor.matmul(sca[0:A, :], kT[j][:, S:SA],
                                 qT[h][:, qs:qe], start=True, stop=True)
                pa = spool.tile([128, QB], MD, tag="pa", bufs=2)
                nc.scalar.activation(pa[0:A, :], sca[0:A, :], Exp,
                                     bias=zb[0:A, :])
                st["pa"] = pa
                return st

            def emit_attn_body(st, uidx):
                h, j, qs, kcs = st["h"], st["j"], st["qs"], st["kcs"]
                ot_ps = ps.tile([128, QB], F32, tag="vproj", bufs=2)
                oa_ps = ps.tile([128, QB], F32, tag="av", bufs=1)
                dr = 64 * (uidx % 2)
                st["ot_ps"], st["oa_ps"], st["dr"] = ot_ps, oa_ps, dr
                groups, gof = st["groups"], st["gof"]
                for ki, kc in enumerate(kcs):
                    pt, c0, q0, N = st["pts"].pop(ki)
                    s0, sp = (ki == 0), (ki == len(kcs) - 1)
                    nc.tensor.matmul(
                        ot_ps[:, q0 - qs:QB],
                        vv[:, kc, j * HD:(j + 1) * HD],
                        pt[:, c0:c0 + N], start=s0, stop=sp)
                    nc.tensor.matmul(
                        dtda[dr:dr + 1, q0 - qs:QB], ones_col[:, 0:1],
                        pt[:, c0:c0 + N], start=s0, stop=sp)
                    gi = gof[ki]
                    if ki == groups[gi][-1]:
                        # sca's slot frees after its exp, so the ring
                        # sustains four outstanding groups mid-unit
                        while (st["next_g"] < len(groups) and
                               st["next_g"] <= gi + 4):
                            st["emit_group"](st["next_g"])
                            st["next_g"] += 1
                # gated adapter denominator (UDT/tanh folded into the
                # contraction column)
                nc.tensor.matmul(dtda[32:33, :], ginv[0:A, h:h + 1],
                                 st["pa"][0:A, :], start=True, stop=True)
                rt = spool.tile([1, QB], F32, tag="rt", bufs=2)
                nc.vector.reciprocal(rt, dtda[dr:dr + 1, :])
                ra = spool.tile([1, QB], F32, tag="ra", bufs=2)
                nc.vector.reciprocal(ra, dtda[32:33, :])
                rtb = spool.tile([128, QB], F32, tag="rtb", bufs=2)
                nc.gpsimd.partition_broadcast(rtb, rt[0:1, :])
                rab = spool.tile([128, QB], F32, tag="rab", bufs=2)
                nc.gpsimd.partition_broadcast(rab, ra[0:1, :])
                st["rtb"], st["rab"] = rtb, rab

            def emit_attn_tail(st):
                h, qh, j = st["h"], st["qh"], st["j"]
                qs, qe = st["qs"], st["qe"]
                pa, ot_ps, oa_ps = st["pa"], st["ot_ps"], st["oa_ps"]
                nc.tensor.matmul(oa_ps[:, :],
                                 vv[0:A, KC, j * HD:(j + 1) * HD],
                                 pa[0:A, :], start=True, stop=True)
                # oT = ot/denom_t + tanh(g)*oa/denom_a  (write-once; the
                # fp8 path emits GAMO-scaled hi/lo e4m3 for the O proj)
                tq1 = spool.tile([128, QB], F32, tag="tq1", bufs=1)
                nc.vector.tensor_mul(tq1, ot_ps[:, :], st["rtb"])
                tq2 = spool.tile([128, QB], F32, tag="tq2", bufs=1)
                nc.vector.tensor_mul(tq2, oa_ps[:, :], st["rab"])
                if fp8:
                    tsum = spool.tile([128, QB], F32, tag="tsum", bufs=1)
                    nc.vector.tensor_add(tsum, tq1, tq2)
                    nc.scalar.copy(oT8[:, 0, h, qs:qe], tsum)
                    nc.vector.tensor_sub(oT8[:, 1, h, qs:qe], tsum,
                                         oT8[:, 0, h, qs:qe])
                else:
                    nc.vector.tensor_add(oT[h][:, qs:qe], tq1, tq2)

            units = [(h, qh) for h in range(HL) for qh in range(NQH)]
            pending = None
            for uidx, (h, qh) in enumerate(units):
                st = emit_attn_prologue(h, qh)
                if deferred_rope is not None:
                    emit_head_finish(deferred_rope, on_dve=True)
                    deferred_rope = None
                if pending is not None:
                    emit_attn_tail(pending)
                emit_attn_body(st, uidx)
                pending = st
            emit_attn_tail(pending)

            # ---------------- phase 3: output projection ------------------
            # wo weights stream through a ring, DMA'd ahead of
            # consumption.  The last output tile is split into halves to
            # shorten the drain tail.
            won = {}

            def load_won(n):
                if n < NB:
                    wt = wpool.tile([128, NV, HL, 512], WD, tag="won",
                                    bufs=4, name=f"won{n}")
                    won[n] = wt
                    nc.sync.dma_start(
                        wt, wop[:, :, :, n * 512:(n + 1) * 512])

            for n in range(min(4, NB)):
                load_won(n)
            for n in range(NB):
                wt = won.pop(n)
                for m in range(NM):
                    last = (n == NB - 1) and (m == NM - 1)
                    halves = ((0, 256), (256, 512)) if last else ((0, 512),)
                    for c0, c1 in halves:
                        pso = ps.tile([128, 512], F32, tag="proj", bufs=4)
                        msl = slice(m * 128, (m + 1) * 128)
                        if fp8:
                            n_ = 0
                            NT = len(TERMS) * (HL // 2)
                            for ov, wv_ in TERMS:
                                for hp in range(HL // 2):
                                    nc.tensor.matmul(
                                        pso[:, 0:c1 - c0],
                                        oT8[:, ov, 2 * hp:2 * hp + 2, msl],
                                        wt[:, wv_, 2 * hp:2 * hp + 2, c0:c1],
                                        start=(n_ == 0), stop=(n_ == NT - 1),
                                        perf_mode=DR)
                                    n_ += 1
                        else:
                            for hh in range(HL):
                                nc.tensor.matmul(
                                    pso[:, 0:c1 - c0],
                                    oT[hh][:, msl],
                                    wt[:, 0, hh, c0:c1],
                                    start=(hh == 0), stop=(hh == HL - 1))
                        ob = obpool.tile([128, 512], MD if fp8 else F32,
                                         tag="ob")
                        if fp8:
                            nc.scalar.mul(ob[:, 0:c1 - c0],
                                          pso[:, 0:c1 - c0], OSC)
                        else:
                            nc.scalar.copy(ob[:, 0:c1 - c0],
                                           pso[:, 0:c1 - c0])
                        nc.sync.dma_start(
                            outp[m, :, n * 512 + c0:n * 512 + c1],
                            ob[:, 0:c1 - c0])
                load_won(n + 4)

    nc.compile()
    nc.finalize()
    return nc


def get_program(KO, S, HL, KVL, causal, mm):
    key = (KO, S, HL, KVL, causal, mm)
    if key not in _PROG_CACHE:
        _PROG_CACHE[key] = build_program(KO, S, HL, KVL, causal, mm)
    return _PROG_CACHE[key]


# --------------------------------------------------------------------------
# host-side sharding / layout prep
# --------------------------------------------------------------------------

_EVEN_FIRST = np.concatenate([np.arange(0, HD, 2), np.arange(1, HD, 2)])


def is_causal_mask(mask):
    S = mask.shape[-1]
    m = np.asarray(mask).reshape(S, S)
    iu = np.triu_indices(S, 1)
    il = np.tril_indices(S)
    return bool(np.all(m[il] == 0.0) and np.all(m[iu] <= -1e8))


def _np_md(mm):
    if mm in ("bf16", "fp8"):
        import ml_dtypes
        return ml_dtypes.bfloat16
    return np.float32


def _np_f8():
    import ml_dtypes
    return ml_dtypes.float8_e4m3


def _q8pair(a32):
    """Split scaled f32 array into e4m3 hi + exact-residual lo."""
    f8 = _np_f8()
    h = np.clip(a32, -240.0, 240.0).astype(f8)
    l = np.clip(a32 - h.astype(np.float32), -240.0, 240.0).astype(f8)
    return h, l


def prep_core_inputs(core, G, x, wq, wk, wv, wo, adapter, gate,
                     freqs_cos, freqs_sin, mask, causal, mm=None):
    """Build the input dict for one core = (batch b, head-group g)."""
    mm = MM_MODE if mm is None else mm
    fp8 = mm == "fp8"
    B, S, D = x.shape
    H = gate.shape[1]
    hd = wq.shape[1] // H
    KV = wk.shape[1] // hd
    KO = D // 128
    KC = S // 128
    HL, KVL = H // G, KV // G
    b, g = core // G, core % G
    hsl = slice(g * HL, (g + 1) * HL)
    ksl = slice(g * KVL, (g + 1) * KVL)
    idx = _EVEN_FIRST
    f32 = np.float32
    md = _np_md(mm)

    def c(a, dt=None):
        return np.ascontiguousarray(a, dtype=dt if dt is not None else md)

    def pairs(key, a32, axis):
        if fp8:
            h, l = _q8pair(np.ascontiguousarray(a32, f32))
            return {key: np.ascontiguousarray(np.stack([h, l], axis=axis))}
        return {key: c(np.expand_dims(a32, axis))}

    inp = {}
    xp = x[b].T.reshape(KO, 128, S).transpose(1, 0, 2)
    inp.update(pairs("xp", xp * AX if fp8 else xp, 1))
    wq4 = wq.reshape(D, H, hd)[:, hsl][:, :, idx] * np.float32(1.0 / np.sqrt(hd))
    wq4 = wq4.reshape(KO, 128, HL, hd).transpose(2, 1, 0, 3)
    inp.update(pairs("wqp", wq4 * AWQ if fp8 else wq4, 2))
    wk4 = wk.reshape(D, KV, hd)[:, ksl][:, :, idx]
    wk4 = wk4.reshape(KO, 128, KVL, hd).transpose(2, 1, 0, 3)
    inp.update(pairs("wkp", wk4 * AWK if fp8 else wk4, 2))
    wv4 = wv.reshape(D, KV, hd)[:, ksl]
    wv4 = wv4.reshape(KO, 128, KVL * hd).transpose(1, 0, 2)
    inp.update(pairs("wvp", wv4 * AWV if fp8 else wv4, 1))
    wos = wo[g * HL * hd:(g + 1) * HL * hd]
    wos = wos.reshape(HL, hd, D).transpose(1, 0, 2)
    inp.update(pairs("wop", wos * AWO if fp8 else wos, 1))
    adx = adapter[0].T.reshape(KO, 128, A).transpose(1, 0, 2)
    adxp = np.zeros((128, KO, AP2), f32)
    adxp[:, :, :A] = adx
    inp.update(pairs("adp", adxp * AX if fp8 else adxp, 1))
    # cos^T / sin^T, duplicated across both partition halves; q rows 0-1
    # carry the q-psum descale, k rows 2-3 the k-psum descale (fp8 mode)
    ct = np.asarray(freqs_cos, dtype=f32).T      # [64, S]
    st = np.asarray(freqs_sin, dtype=f32).T
    sq = 1.0 / A1Q if fp8 else 1.0
    sk = 1.0 / A1K if fp8 else 1.0
    csp = np.empty((128, 4, S), f32)
    for half in (slice(0, 64), slice(64, 128)):
        csp[half, 0] = ct * sq
        csp[half, 1] = st * sq
        csp[half, 2] = ct * sk
        csp[half, 3] = st * sk
    csp = c(csp)
    triu = np.triu(np.ones((128, 128), dtype=f32))
    tri = np.empty((128, 3, 128), f32)
    tri[:, 0] = (UDT if fp8 else 1.0) * np.ones((128, 128), f32)
    tri[:, 1] = np.eye(128, dtype=f32)
    tri[:, 2] = -1e9 * (1.0 - triu)
    tri = c(tri)
    gth = np.tanh(np.asarray(gate[0, hsl, 0, 0], dtype=np.float64)).astype(f32)
    with np.errstate(divide="ignore"):
        gi = (UDT if fp8 else 1.0) / gth
        ginv = np.broadcast_to(gi.reshape(1, HL), (128, HL))
    ginvp = c(ginv)
    inp.update({"csp": csp, "trip": tri, "ginvp": ginvp})
    if not causal:
        mt = np.asarray(mask).reshape(S, S).T  # [keys, q]
        inp["mtp"] = c(mt.reshape(KC, 128, S).transpose(1, 0, 2), f32)
    return inp


# --------------------------------------------------------------------------
# entry point
# --------------------------------------------------------------------------

def kernel(x, wq, wk, wv, wo, adapter, gate, freqs_cos, freqs_sin, mask,
           _trace=False):
    x, wq, wk, wv, wo, adapter, gate, freqs_cos, freqs_sin, mask = (
        np.asarray(a) for a in
        (x, wq, wk, wv, wo, adapter, gate, freqs_cos, freqs_sin, mask))
    B, S, D = x.shape
    H = gate.shape[1]
    hd = wq.shape[1] // H
    KV = wk.shape[1] // hd
    G = 8 // B                      # head groups per batch over 8 cores
    HL, KVL = H // G, KV // G
    KO = D // 128

    causal = is_causal_mask(mask)
    nc = get_program(KO, S, HL, KVL, causal, MM_MODE)

    in_maps = [prep_core_inputs(core, G, x, wq, wk, wv, wo, adapter, gate,
                                freqs_cos, freqs_sin, mask, causal)
               for core in range(8)]
    res = run_bass_kernel_spmd(nc, in_maps, core_ids=list(range(8)),
                               trace=_trace)
    out = np.zeros((B, S, D), np.float32)
    for core in range(8):
        b = core // G
        r = res.results[core]
        out[b] += np.asarray(r["out"], np.float32).reshape(S, D)
    if _trace:
        kernel._last_result = res
    return out
